# revision 1
# baseline (speedup 1.0000x reference)
"""DCGRU cell (nn_DCGRUCell) Trainium2 Bass kernel, 8 NeuronCores.

Sharding: node dimension N=4096 split 8 ways (512 rows/core); supports are
fed host-transposed (T = A^T) so tensor-engine matmuls need no on-device
transposes of A. Hop-1 diffusion products are computed node-major
[node, (batch, feat)] and AllGathered across cores; hop-2 products are
computed directly in transposed (feature-major) form since they only feed
the dense W stage. All matmuls run fp32 (float32r streaming); PSUM fp32.

kernel(**inputs) takes the FULL inputs from reference.setup_inputs() and
returns the FULL [16, 4096, 64] float32 output.
"""
import os
import numpy as np

import concourse.bass as bass
import concourse.mybir as mybir
import concourse.tile as tile
from concourse import bacc
from concourse.bass_utils import run_bass_kernel_spmd

F32 = mybir.dt.float32
F32R = mybir.dt.float32r
AF = mybir.ActivationFunctionType

NCORES = 8
B, N, H, DIN = 16, 4096, 64, 2
C = DIN + H                 # 66 features per batch into each GCN
BC = B * C                  # 1056
NOWN = N // NCORES          # 512 rows per core
NT = NOWN // 128            # 4 n-tiles per core
MT = N // 128               # 32 m-tiles (contraction)
MAIN = 1024                 # bc columns in the node-major main sweep
RAG = BC - MAIN             # 32 ragged columns
JT = BC // 128              # 8 full 128-col feature tiles (+1 ragged)
MAIN_ELEMS = NOWN * MAIN
RAG_ELEMS = NT * 128 * RAG
SHARD = MAIN_ELEMS + RAG_ELEMS
GROUP = [list(range(NCORES))]

_NC_CACHE = {}


def build_nc():
    nc = bacc.Bacc("TRN2", target_bir_lowering=False, debug=False,
                   num_devices=NCORES)

    d = {}
    d["Ts"] = nc.dram_tensor("Ts", [2, N, NOWN], F32R, kind="ExternalInput")
    d["xs_main"] = nc.dram_tensor("xs_main", [N, MAIN], F32R,
                                  kind="ExternalInput")
    d["xs_rag"] = nc.dram_tensor("xs_rag", [MT, 128, RAG], F32R,
                                 kind="ExternalInput")
    d["xsT_orig"] = nc.dram_tensor("xsT_orig", [BC, NOWN], F32R,
                                   kind="ExternalInput")
    d["xsT_own"] = nc.dram_tensor("xsT_own", [BC, NOWN], F32R,
                                  kind="ExternalInput")
    d["Wg"] = nc.dram_tensor("Wg", [5 * C, 2 * H], F32R, kind="ExternalInput")
    d["bg"] = nc.dram_tensor("bg", [2 * H, 1], F32, kind="ExternalInput")
    d["Wu"] = nc.dram_tensor("Wu", [5 * C, H], F32R, kind="ExternalInput")
    d["bu"] = nc.dram_tensor("bu", [H, 1], F32, kind="ExternalInput")
    d["negI"] = nc.dram_tensor("negI", [128, 128], F32R, kind="ExternalInput")
    d["outT"] = nc.dram_tensor("outT", [B, H, NOWN], F32,
                               kind="ExternalOutput")

    with tile.TileContext(nc) as tc:
        _emit(nc, tc, d)
    nc.compile()
    return nc


def _emit(nc, tc, d):
    import contextlib
    stack = contextlib.ExitStack()
    with stack:
        const = stack.enter_context(tc.tile_pool(name="const", bufs=1))
        sb_ex = stack.enter_context(tc.tile_pool(name="ex", bufs=1))
        sb_mov = stack.enter_context(tc.tile_pool(name="mov", bufs=1))
        sb_sm = stack.enter_context(tc.tile_pool(name="small", bufs=1))
        dram = stack.enter_context(
            tc.tile_pool(name="dram", bufs=1, space="DRAM"))
        psum = stack.enter_context(
            tc.tile_pool(name="psum", bufs=1, space="PSUM"))

        # ---- constants / resident tensors ----
        # supports loaded in interleaved 4-m-tile chunks so the first
        # matmuls only wait for the first small chunk
        CH = 4
        NCH = MT // CH
        Tch = {}
        for s in range(2):
            for k in range(NCH):
                Tch[(s, k)] = const.tile([128, CH, 512], F32R,
                                         name=f"T{s}_{k}")
        for k in range(NCH):
            for s in range(2):
                ts = d["Ts"].ap()[s].rearrange("(t p) n -> p t n", p=128)
                nc.sync.dma_start(Tch[(s, k)][:],
                                  ts[:, k * CH:(k + 1) * CH, :])

        def T_tile(s, m):
            return Tch[(s, m // CH)][:, m % CH, :]

        ident = const.tile([128, 128], F32)
        nc.gpsimd.memset(ident[:], 0.0)
        nc.gpsimd.affine_select(
            out=ident[:], in_=ident[:],
            compare_op=mybir.AluOpType.not_equal, fill=1.0, base=0,
            pattern=[[-1, 128]], channel_multiplier=1)
        nident = const.tile([128, 128], F32R)  # -0.5*I (for 2*(Ax - X/2))
        nc.sync.dma_start(nident[:], d["negI"].ap())

        wg_t = const.tile([C, 5, 2 * H], F32R)
        wu_t = const.tile([C, 5, H], F32R)
        for j in range(5):
            nc.sync.dma_start(wg_t[:, j, :],
                              d["Wg"].ap()[j * C:(j + 1) * C, :])
            nc.sync.dma_start(wu_t[:, j, :],
                              d["Wu"].ap()[j * C:(j + 1) * C, :])
        bg_t = const.tile([2 * H, 1], F32)
        nc.sync.dma_start(bg_t[:], d["bg"].ap())
        bu_t = const.tile([H, 1], F32)
        nc.sync.dma_start(bu_t[:], d["bu"].ap())

        # ---- DRAM staging ----
        # AG slots: 0,1 = y1 of gcn1; 2 = cand; 3,4 = y1 of gcn2
        ag_in = [dram.tile([SHARD], F32R, name=f"agin{i}") for i in range(5)]
        ag_out = [dram.tile([NCORES * SHARD], F32R, name=f"agout{i}",
                            addr_space="Shared") for i in range(5)]
        yt = [dram.tile([BC, NOWN], F32R, name=f"yt{i}") for i in range(4)]
        yt2 = [dram.tile([BC, NOWN], F32R, name=f"yt2_{i}") for i in range(4)]
        candT_dram = dram.tile([BC, NOWN], F32R)
        rt_dram = dram.tile([B, H, NOWN], F32)

        def xs_main_half(m, h):
            return d["xs_main"].ap()[m * 128:(m + 1) * 128,
                                     h * 512:(h + 1) * 512]

        def xs_rag_tile(m):
            return d["xs_rag"].ap()[m]

        def ag_main_half(i, m, h):
            a = ag_out[i].opt()
            off = (m // NT) * SHARD + (m % NT) * 128 * MAIN
            v = a[off:off + 128 * MAIN].rearrange("(p f) -> p f", f=MAIN)
            return v[:, h * 512:(h + 1) * 512]

        def ag_rag_tile(i, m):
            a = ag_out[i].opt()
            off = (m // NT) * SHARD + MAIN_ELEMS + (m % NT) * 128 * RAG
            return a[off:off + 128 * RAG].rearrange("(p f) -> p f", f=RAG)

        def ag_col_chunk(i, q, j):
            """[128, NT, 128] moving chunk: rank q's own rows, feature
            columns 128j..128(j+1)."""
            a = ag_out[i].opt()
            blk = a[q * SHARD:q * SHARD + MAIN_ELEMS].rearrange(
                "(t p f) -> p t f", p=128, f=MAIN)
            return blk[:, :, j * 128:(j + 1) * 128]

        def agin_own_main(i, t):
            a = ag_in[i].opt()
            return a[t * 128 * MAIN:(t + 1) * 128 * MAIN].rearrange(
                "(p f) -> p f", f=MAIN)

        def agin_own_main_half(i, t, h):
            return agin_own_main(i, t)[:, h * 512:(h + 1) * 512]

        def agin_own_rag(i, t):
            a = ag_in[i].opt()
            off = MAIN_ELEMS + t * 128 * RAG
            return a[off:off + 128 * RAG].rearrange("(p f) -> p f", f=RAG)

        # ============ hop-1 products: node-major + AllGather ============
        # Both supports share one pass over the moving operand.
        def emit_hop1_pair(pid, mov_main, mov_rag, agin_idx, yt_dst):
            """Y1_s[own rows, :] = A_s @ M for s in (0, 1)."""
            # ragged columns, transposed: psum[c(32), n(512)] per support
            ps_t = [psum.tile([RAG, NOWN], F32, name=f"pst{pid}{s}",
                              tag="acc", bufs=8) for s in range(2)]
            for m in range(MT):
                mvr = sb_mov.tile([128, RAG], F32R, name=f"mvr{pid}_{m}",
                                  tag="movr", bufs=8)
                nc.sync.dma_start(mvr[:], mov_rag(m))
                for s in range(2):
                    nc.tensor.matmul(ps_t[s][:], mvr[:], T_tile(s, m),
                                     start=(m == 0), stop=(m == MT - 1))
            for s in range(2):
                rag_ex = sb_sm.tile([RAG, NOWN], F32, name=f"rgex{pid}{s}",
                                    tag="ragex", bufs=1)
                nc.vector.tensor_copy(rag_ex[:], ps_t[s][:])
                nc.sync.dma_start(yt_dst[s].opt()[MAIN:BC, :].bitcast(F32),
                                  rag_ex[:])
                for t in range(NT):
                    tp = psum.tile([128, RAG], F32, name=f"rtp{pid}{s}",
                                   tag="acc", bufs=8)
                    nc.tensor.transpose(
                        tp[:], rag_ex[:, t * 128:(t + 1) * 128],
                        ident[0:RAG, 0:RAG])
                    rnm = sb_sm.tile([128, RAG], F32, name=f"rnm{pid}{s}",
                                     tag="rnm", bufs=2)
                    nc.vector.tensor_copy(rnm[:], tp[:])
                    nc.sync.dma_start(
                        agin_own_rag(agin_idx[s], t).bitcast(F32), rnm[:])

            # main columns in two 512-wide sweeps; 2 supports x 4 n-tiles
            # of accumulators fill all 8 PSUM banks per sweep
            for hh in range(2):
                ps_m = {}
                for s in range(2):
                    for n in range(NT):
                        ps_m[(s, n)] = psum.tile(
                            [128, 512], F32, name=f"psm{pid}_{hh}{s}{n}",
                            tag="acc", bufs=8)
                for m in range(MT):
                    mv = sb_mov.tile([128, 512], F32R,
                                     name=f"mv{pid}_{hh}_{m}", tag="mov",
                                     bufs=4)
                    nc.sync.dma_start(mv[:], mov_main(m, hh))
                    for s in range(2):
                        for n in range(NT):
                            nc.tensor.matmul(
                                ps_m[(s, n)][:],
                                T_tile(s, m)[:, n * 128:(n + 1) * 128],
                                mv[:], start=(m == 0), stop=(m == MT - 1))
                for s in range(2):
                    exhs = []
                    for n in range(NT):
                        exh = sb_ex.tile([128, 512], F32,
                                         name=f"ex{pid}{hh}{s}{n}",
                                         tag="ex", bufs=6)
                        nc.vector.tensor_copy(exh[:], ps_m[(s, n)][:])
                        nc.sync.dma_start(
                            agin_own_main_half(agin_idx[s], n, hh)
                            .bitcast(F32), exh[:])
                        exhs.append(exh)
                    # feature-major staging: per bc row-block j, transpose
                    # the 4 n-chunks and write one contiguous row-block
                    for j in range(4):
                        st4 = sb_sm.tile([128, NOWN], F32,
                                         name=f"st4{pid}", tag="st", bufs=2)
                        for n in range(NT):
                            tp = psum.tile([128, 128], F32,
                                           name=f"tp{pid}", tag="acc",
                                           bufs=8)
                            nc.tensor.transpose(
                                tp[:], exhs[n][:, j * 128:(j + 1) * 128],
                                ident[:])
                            nc.vector.tensor_copy(
                                st4[:, n * 128:(n + 1) * 128], tp[:])
                        jj = hh * 4 + j
                        nc.sync.dma_start(
                            yt_dst[s].opt()[jj * 128:(jj + 1) * 128, :]
                            .bitcast(F32), st4[:])
            nc.gpsimd.collective_compute(
                "AllGather", mybir.AluOpType.bypass, replica_groups=GROUP,
                ins=[ag_in[agin_idx[0]].opt()],
                outs=[ag_out[agin_idx[0]].opt()])
            nc.gpsimd.collective_compute(
                "AllGather", mybir.AluOpType.bypass, replica_groups=GROUP,
                ins=[ag_in[agin_idx[1]].opt()],
                outs=[ag_out[agin_idx[1]].opt()])

        # ======= hop-2 product: transposed form (feature-major out) =======
        def emit_hop2(pid, s, ag_idx, ownT_rows, yt_dst):
            """Y2^T[bc, own n] = 2*(A_s @ Y1)^T[bc, n] - X^T[bc, n].

            Moving operand = gathered Y1 (ag_out[ag_idx]) loaded as full
            m-rows; its 128-col slices act as lhsT for 8 concurrent
            feature-tile accumulators. ownT_rows(j, w) gives X^T rows for
            the -X term."""
            # ragged feature tile (j = JT), its own accumulation
            ps_r = psum.tile([RAG, NOWN], F32, name=f"ph2r{pid}", tag="acc",
                             bufs=8)
            for m in range(MT):
                mvr = sb_mov.tile([128, RAG], F32R, name=f"mvr{pid}_{m}",
                                  tag="movr", bufs=8)
                nc.sync.dma_start(mvr[:], ag_rag_tile(ag_idx, m))
                nc.tensor.matmul(ps_r[:], mvr[:], T_tile(s, m),
                                 start=(m == 0), stop=False)
            xrt = sb_mov.tile([RAG, NOWN], F32R, name=f"xrt{pid}r",
                              tag="xrt", bufs=2)
            nc.sync.dma_start(xrt[:], ownT_rows(JT, RAG))
            nc.tensor.matmul(ps_r[:], nident[0:RAG, 0:RAG], xrt[:],
                             start=False, stop=True)
            exr = sb_ex.tile([RAG, NOWN], F32, name=f"h2exr{pid}",
                             tag="ex", bufs=6)
            nc.scalar.mul(exr[:], ps_r[:], 2.0)
            nc.sync.dma_start(
                yt_dst.opt()[MAIN:BC, :].bitcast(F32), exr[:])

            # 8 full feature tiles, m-outer (row loads are contiguous)
            ps = [psum.tile([128, NOWN], F32, name=f"ph2{pid}_{j}",
                            tag="acc", bufs=8) for j in range(JT)]
            for m in range(MT):
                mrow = sb_mov.tile([128, MAIN], F32R, name=f"mr{pid}_{m}",
                                   tag="mov", bufs=4)
                for h in range(2):
                    nc.sync.dma_start(mrow[:, h * 512:(h + 1) * 512],
                                      ag_main_half(ag_idx, m, h))
                for j in range(JT):
                    nc.tensor.matmul(
                        ps[j][:], mrow[:, j * 128:(j + 1) * 128],
                        T_tile(s, m), start=(m == 0), stop=False)
            for j in range(JT):
                xrt = sb_mov.tile([128, NOWN], F32R, name=f"xrt{pid}_{j}",
                                  tag="xrt", bufs=2)
                nc.sync.dma_start(xrt[:], ownT_rows(j, 128))
                nc.tensor.matmul(ps[j][:], nident[:], xrt[:],
                                 start=False, stop=True)
                exh = sb_ex.tile([128, NOWN], F32, name=f"h2ex{pid}_{j}",
                                 tag="ex", bufs=6)
                nc.scalar.mul(exh[:], ps[j][:], 2.0)
                nc.sync.dma_start(
                    yt_dst.opt()[j * 128:(j + 1) * 128, :].bitcast(F32),
                    exh[:])

        # ======================= GCN 1 (gate) =======================
        emit_hop1_pair("g1h1", xs_main_half, xs_rag_tile, (0, 1),
                       (yt[0], yt[2]))

        def xsT_orig_rows(j, w):
            return d["xsT_orig"].ap()[j * 128:j * 128 + w, :]

        emit_hop2("g1s0h2", 0, 0, xsT_orig_rows, yt[1])
        emit_hop2("g1s1h2", 1, 1, xsT_orig_rows, yt[3])

        # gate W-stage + candidate build
        for b in range(B):
            xsT_b = sb_sm.tile([C, NOWN], F32R, name="xsTb", tag="xsTb",
                               bufs=2)
            nc.sync.dma_start(xsT_b[:],
                              d["xsT_own"].ap()[b * C:(b + 1) * C, :])
            blocks = [xsT_b]
            for j in range(4):
                bt = sb_sm.tile([C, NOWN], F32R, name=f"blk{j}",
                                tag=f"blk{j}", bufs=2)
                nc.sync.dma_start(bt[:], yt[j].opt()[b * C:(b + 1) * C, :])
                blocks.append(bt)
            zr_ps = psum.tile([2 * H, NOWN], F32, name="zrps", tag="acc", bufs=8)
            for j in range(5):
                nc.tensor.matmul(zr_ps[:], wg_t[:, j, :], blocks[j][:],
                                 start=(j == 0), stop=(j == 4))
            zr = sb_sm.tile([2 * H, NOWN], F32, name="zr", tag="zr", bufs=1)
            nc.scalar.activation(zr[:], zr_ps[:], AF.Sigmoid, bias=bg_t[:])
            nc.sync.dma_start(rt_dram.opt()[b], zr[H:2 * H, :])
            # candT_b rows are [z*state(64); x(2)] (host permutes W rows)
            cT = sb_sm.tile([C, NOWN], F32, name="cT", tag="cT", bufs=1)
            nc.vector.tensor_mul(cT[0:H, :], zr[0:H, :],
                                 xsT_b[0:H, :].bitcast(F32))
            nc.vector.tensor_copy(cT[H:C, :], xsT_b[H:C, :].bitcast(F32))
            nc.sync.dma_start(
                candT_dram.opt()[b * C:(b + 1) * C, :].bitcast(F32), cT[:])
            # cand node-major -> ag_in[2]
            a_main = ag_in[2].opt()[0:MAIN_ELEMS].rearrange(
                "(p f) -> p f", f=MAIN).bitcast(F32)
            for t in range(NT):
                ps = psum.tile([128, C], F32, name="ctps", tag="acc", bufs=8)
                nc.tensor.transpose(ps[:], cT[:, t * 128:(t + 1) * 128],
                                    ident[0:C, 0:C])
                ct_nm = sb_sm.tile([128, C], F32, name="ctnm", tag="ctnm", bufs=1)
                nc.vector.tensor_copy(ct_nm[:], ps[:])
                lo, hi = b * C, (b + 1) * C
                if hi <= MAIN:
                    nc.sync.dma_start(
                        a_main[t * 128:(t + 1) * 128, lo:hi], ct_nm[:])
                else:
                    cut = MAIN - lo
                    nc.sync.dma_start(
                        a_main[t * 128:(t + 1) * 128, lo:MAIN],
                        ct_nm[:, 0:cut])
                    nc.sync.dma_start(agin_own_rag(2, t).bitcast(F32),
                                      ct_nm[:, cut:C])
        nc.gpsimd.collective_compute(
            "AllGather", mybir.AluOpType.bypass, replica_groups=GROUP,
            ins=[ag_in[2].opt()], outs=[ag_out[2].opt()])

        # ======================= GCN 2 (update) =======================
        emit_hop1_pair("g2h1",
                       lambda m, h: ag_main_half(2, m, h),
                       lambda m: ag_rag_tile(2, m), (3, 4),
                       (yt2[0], yt2[2]))

        def candT_rows(j, w):
            return candT_dram.opt()[j * 128:j * 128 + w, :]

        emit_hop2("g2s0h2", 0, 3, candT_rows, yt2[1])
        emit_hop2("g2s1h2", 1, 4, candT_rows, yt2[3])

        # update W-stage + final combine
        for b in range(B):
            cT_b = sb_sm.tile([C, NOWN], F32R, name="cTb", tag="xsTb",
                              bufs=2)
            nc.sync.dma_start(cT_b[:],
                              candT_dram.opt()[b * C:(b + 1) * C, :])
            blocks = [cT_b]
            for j in range(4):
                bt = sb_sm.tile([C, NOWN], F32R, name=f"ublk{j}",
                                tag=f"blk{j}", bufs=2)
                nc.sync.dma_start(bt[:], yt2[j].opt()[b * C:(b + 1) * C, :])
                blocks.append(bt)
            hc_ps = psum.tile([H, NOWN], F32, name="hcps", tag="acc", bufs=8)
            for j in range(5):
                nc.tensor.matmul(hc_ps[:], wu_t[:, j, :], blocks[j][:],
                                 start=(j == 0), stop=(j == 4))
            hc = sb_sm.tile([H, NOWN], F32, name="hc", tag="zr", bufs=1)
            nc.scalar.activation(hc[:], hc_ps[:], AF.Tanh, bias=bu_t[:])

            # out = hc + r * (state - hc);  stateT = xsT_own rows [0:H]
            xsT_b = sb_sm.tile([C, NOWN], F32, name="xsTb2", tag="cT",
                               bufs=1)
            nc.sync.dma_start(
                xsT_b[:],
                d["xsT_own"].ap()[b * C:(b + 1) * C, :].bitcast(F32))
            rT = sb_sm.tile([H, NOWN], F32, name="rT", tag="rT", bufs=1)
            nc.sync.dma_start(rT[:], rt_dram.opt()[b])
            tmp = sb_sm.tile([H, NOWN], F32, name="tmp", tag="tmp", bufs=2)
            nc.vector.tensor_sub(tmp[:], xsT_b[0:H, :], hc[:])
            nc.vector.tensor_mul(tmp[:], rT[:], tmp[:])
            ot = sb_sm.tile([H, NOWN], F32, name="ot", tag="ot", bufs=2)
            nc.vector.tensor_add(ot[:], hc[:], tmp[:])
            nc.sync.dma_start(d["outT"].ap()[b], ot[:])


def prepare_in_maps(x, state, support0, support1, W_gate, b_gate,
                    W_update, b_update):
    xs = np.concatenate([x, state], axis=-1)          # [B, N, C]
    xs_nm = np.ascontiguousarray(
        xs.transpose(1, 0, 2).reshape(N, BC)).astype(np.float32)
    # feature-major input for W / elementwise uses [state(64); x(2)] rows
    sx_nm = np.ascontiguousarray(
        np.concatenate([state, x], axis=-1)
        .transpose(1, 0, 2).reshape(N, BC)).astype(np.float32)
    perm = np.r_[DIN:C, 0:DIN]                 # [x, state] -> [state, x]
    Wg_dev = np.ascontiguousarray(W_gate, dtype=np.float32).copy()
    Wg_dev[0:C] = Wg_dev[0:C][perm]            # only the X-block reads xsT
    Wu_dev = np.ascontiguousarray(W_update, dtype=np.float32).copy()
    for j in range(5):                         # all of cand's blocks permute
        Wu_dev[j * C:(j + 1) * C] = Wu_dev[j * C:(j + 1) * C][perm]
    xs_main = np.ascontiguousarray(xs_nm[:, :MAIN])
    xs_rag = np.ascontiguousarray(xs_nm[:, MAIN:]).reshape(MT, 128, RAG)
    bg = np.ascontiguousarray(b_gate, dtype=np.float32).reshape(2 * H, 1)
    bu = np.ascontiguousarray(b_update, dtype=np.float32).reshape(H, 1)
    negI = (-0.5 * np.eye(128, dtype=np.float32))

    in_maps = []
    for r in range(NCORES):
        n0 = r * NOWN
        sl = xs_nm[n0:n0 + NOWN]
        in_maps.append({
            "Ts": np.ascontiguousarray(
                np.stack([support0[n0:n0 + NOWN, :].T,
                          support1[n0:n0 + NOWN, :].T])).astype(np.float32),
            "xs_main": xs_main,
            "xs_rag": xs_rag,
            "xsT_orig": np.ascontiguousarray(sl.T),
            "xsT_own": np.ascontiguousarray(sx_nm[n0:n0 + NOWN].T),
            "Wg": Wg_dev, "bg": bg, "Wu": Wu_dev, "bu": bu,
            "negI": negI,
        })
    return in_maps


def assemble_output(results):
    out = np.empty((B, N, H), dtype=np.float32)
    for r in range(NCORES):
        n0 = r * NOWN
        out[:, n0:n0 + NOWN, :] = results[r]["outT"].transpose(0, 2, 1)
    return out


def get_nc():
    if "nc" not in _NC_CACHE:
        _NC_CACHE["nc"] = build_nc()
    return _NC_CACHE["nc"]


def kernel(x, state, support0, support1, W_gate, b_gate, W_update, b_update):
    nc = get_nc()
    in_maps = prepare_in_maps(x, state, support0, support1,
                              W_gate, b_gate, W_update, b_update)
    prev = os.environ.get("BASS_NEVER_TRACE")
    os.environ["BASS_NEVER_TRACE"] = "1"
    try:
        res = run_bass_kernel_spmd(nc, in_maps, list(range(NCORES)),
                                   trace=False)
    finally:
        if prev is None:
            os.environ.pop("BASS_NEVER_TRACE", None)
        else:
            os.environ["BASS_NEVER_TRACE"] = prev
    return assemble_output(res.results)



# revision 15
# speedup vs baseline: 1.5106x; 1.5106x over previous
"""DCGRU cell (nn_DCGRUCell) Trainium2 Bass kernel, 8 NeuronCores — v2.

Sharding: node dim N=4096 split 8 ways (512/core); supports resident in
SBUF as bf16 (A^T slices). All diffusion matmuls bf16 with fp32 PSUM.

Node-major moving layout splits columns [B*H=1024 state | B*2=32 x]; the
x-feature diffusion is identical for both GCNs and computed once. Hop-1
outputs AllGather in two column halves so each collective hides under
the next compute phase; hop-2 outputs and all W-stage operands stay in
SBUF (feature-major Y1 blocks arrive via DMA-transpose reads of the
locally-written AG input). W stationaries are duplicated across both
partition halves so per-batch [64,512] moving slices at partition 64
are legal; the 10 x-feature rows per batch sit in 32-row-aligned slots
of a packed XALL tile (row-tiled K=10 matmul).

kernel(**inputs) takes FULL inputs, returns FULL [16,4096,64] fp32.
"""
import os
import numpy as np

import concourse.bass as bass
import concourse.mybir as mybir
import concourse.tile as tile
from concourse import bacc
from concourse.bass_utils import run_bass_kernel_spmd

F32 = mybir.dt.float32
BF16 = mybir.dt.bfloat16
AF = mybir.ActivationFunctionType

NCORES = 8
B, N, H, DIN = 16, 4096, 64, 2
NOWN = N // NCORES          # 512 rows per core
NT = NOWN // 128            # 4 n-tiles per core
MT = N // 128               # 32 m-tiles (contraction)
MAIN = B * H                # 1024 state columns, batch-major
RAG = B * DIN               # 32 x columns, batch-major
HALF = 512                  # column half for AG chunking
SH_MAIN = NOWN * HALF       # elems of one support's half in a shard
SH_A = 2 * SH_MAIN          # chunk-A shard elems (both supports)
SH_RAGS = NOWN * RAG        # one support's ragged elems
SH_B = SH_A + 2 * SH_RAGS   # chunk-B shard elems
SH_C = NOWN * HALF          # cand chunk shard elems
GROUP = [list(range(NCORES))]

_NC_CACHE = {}


def build_nc():
    nc = bacc.Bacc("TRN2", target_bir_lowering=False, debug=False,
                   num_devices=NCORES)
    d = {}
    d["Ts"] = nc.dram_tensor("Ts", [2, N, NOWN], BF16, kind="ExternalInput")
    d["xs_main"] = nc.dram_tensor("xs_main", [N, MAIN], BF16,
                                  kind="ExternalInput")
    d["xs_rag"] = nc.dram_tensor("xs_rag", [N, RAG], BF16,
                                 kind="ExternalInput")
    d["stateT"] = nc.dram_tensor("stateT", [8, 128, NOWN], BF16,
                                 kind="ExternalInput")
    d["xT"] = nc.dram_tensor("xT", [RAG, NOWN], BF16, kind="ExternalInput")
    d["P1"] = nc.dram_tensor("P1", [4, 96, 128], BF16, kind="ExternalInput")
    d["P2"] = nc.dram_tensor("P2", [4, 64, 128], BF16, kind="ExternalInput")
    d["Wg_main"] = nc.dram_tensor("Wg_main", [5, 128, 128], BF16,
                                  kind="ExternalInput")
    d["Wg_x"] = nc.dram_tensor("Wg_x", [128, 128], BF16,
                               kind="ExternalInput")
    d["Wu_main"] = nc.dram_tensor("Wu_main", [5, 128, H], BF16,
                                  kind="ExternalInput")
    d["Wu_x"] = nc.dram_tensor("Wu_x", [128, H], BF16, kind="ExternalInput")
    d["bg"] = nc.dram_tensor("bg", [2 * H, 1], F32, kind="ExternalInput")
    d["bu"] = nc.dram_tensor("bu", [H, 1], F32, kind="ExternalInput")
    d["negI"] = nc.dram_tensor("negI", [128, 128], BF16,
                               kind="ExternalInput")
    d["identb"] = nc.dram_tensor("identb", [128, 128], BF16,
                                 kind="ExternalInput")
    d["outT"] = nc.dram_tensor("outT", [B, H, NOWN], F32,
                               kind="ExternalOutput")
    with tile.TileContext(nc) as tc:
        _emit(nc, tc, d)
    nc.compile()
    return nc


def _emit(nc, tc, d):
    import contextlib
    stack = contextlib.ExitStack()
    with stack:
        const = stack.enter_context(tc.tile_pool(name="const", bufs=1))
        res = stack.enter_context(tc.tile_pool(name="res", bufs=1))
        sb_mov = stack.enter_context(tc.tile_pool(name="mov", bufs=1))
        sb_ex = stack.enter_context(tc.tile_pool(name="ex", bufs=1))
        sb_y1 = stack.enter_context(tc.tile_pool(name="y1f", bufs=1))
        sb_y2 = stack.enter_context(tc.tile_pool(name="y2", bufs=1))
        sb_sm = stack.enter_context(tc.tile_pool(name="small", bufs=1))
        dram = stack.enter_context(
            tc.tile_pool(name="dram", bufs=1, space="DRAM"))
        psum = stack.enter_context(
            tc.tile_pool(name="psum", bufs=1, space="PSUM"))

        # ---------------- resident tensors ----------------
        CH = 4
        NCH = MT // CH
        Tch = {}
        for s in range(2):
            for k in range(NCH):
                Tch[(s, k)] = const.tile([128, CH, NOWN], BF16,
                                         name=f"T{s}_{k}")
        for k in range(NCH):
            for s in range(2):
                ts = d["Ts"].ap()[s].rearrange("(t p) n -> p t n", p=128)
                nc.sync.dma_start(Tch[(s, k)][:],
                                  ts[:, k * CH:(k + 1) * CH, :])

        def T_tile(s, m):
            return Tch[(s, m // CH)][:, m % CH, :]

        negI = const.tile([128, 128], BF16)
        nc.sync.dma_start(negI[:], d["negI"].ap())
        identb = const.tile([128, 128], BF16)
        nc.sync.dma_start(identb[:], d["identb"].ap())
        # S1 rows: [x^T(32); y1x_s0^T(32); y1x_s1^T(32)]; S2: y2x per support
        S1 = res.tile([96, NOWN], BF16, name="S1")
        nc.sync.dma_start(S1[0:RAG, :], d["xT"].ap())
        S2 = res.tile([64, NOWN], BF16, name="S2")
        P1 = const.tile([96, 4, 128], BF16)
        P2 = const.tile([64, 4, 128], BF16)
        for g in range(4):
            nc.sync.dma_start(P1[:, g, :], d["P1"].ap()[g])
            nc.sync.dma_start(P2[:, g, :], d["P2"].ap()[g])

        stateT = [res.tile([128, NOWN], BF16, name=f"stT{p}")
                  for p in range(8)]
        for p in range(8):
            nc.sync.dma_start(stateT[p][:], d["stateT"].ap()[p])
        xall = [res.tile([128, NOWN], BF16, name=f"xall{g}")
                for g in range(4)]

        wg_m = const.tile([128, 5, 128], BF16)
        wu_m = const.tile([128, 5, H], BF16)
        for j in range(5):
            nc.sync.dma_start(wg_m[:, j, :], d["Wg_main"].ap()[j])
            nc.sync.dma_start(wu_m[:, j, :], d["Wu_main"].ap()[j])
        wg_x = const.tile([128, 128], BF16)
        nc.sync.dma_start(wg_x[:], d["Wg_x"].ap())
        wu_x = const.tile([128, H], BF16)
        nc.sync.dma_start(wu_x[:], d["Wu_x"].ap())
        bg_t = const.tile([2 * H, 1], F32)
        nc.sync.dma_start(bg_t[:], d["bg"].ap())
        bu_t = const.tile([H, 1], F32)
        nc.sync.dma_start(bu_t[:], d["bu"].ap())

        rt = [res.tile([128, NOWN], BF16, name=f"rt{p}") for p in range(8)]
        zs = [res.tile([128, NOWN], BF16, name=f"zs{p}") for p in range(8)]

        # ---------------- DRAM staging ----------------
        agin1A = dram.tile([SH_A], BF16, name="agin1A")
        agin1B = dram.tile([SH_B], BF16, name="agin1B")
        agout1A = dram.tile([NCORES * SH_A], BF16, name="agout1A",
                            addr_space="Shared")
        agout1B = dram.tile([NCORES * SH_B], BF16, name="agout1B",
                            addr_space="Shared")
        aginC = [dram.tile([SH_C], BF16, name=f"aginC{h}") for h in range(2)]
        agoutC = [dram.tile([NCORES * SH_C], BF16, name=f"agoutC{h}",
                            addr_space="Shared") for h in range(2)]
        agin2A = dram.tile([SH_A], BF16, name="agin2A")
        agin2B = dram.tile([SH_A], BF16, name="agin2B")
        agout2A = dram.tile([NCORES * SH_A], BF16, name="agout2A",
                            addr_space="Shared")
        agout2B = dram.tile([NCORES * SH_A], BF16, name="agout2B",
                            addr_space="Shared")

        def agin_main(buf, s, t):
            """[128, 512] own node-tile t of support s (main half)."""
            a = buf.opt()
            off = s * SH_MAIN + t * 128 * HALF
            return a[off:off + 128 * HALF].rearrange("(p f) -> p f", f=HALF)

        def agin_view(buf, s):
            """[512, 512] node-major own rows of support s."""
            a = buf.opt()
            return a[s * SH_MAIN:(s + 1) * SH_MAIN].rearrange(
                "(n c) -> n c", c=HALF)

        def agin_rag(s, t):
            a = agin1B.opt()
            off = SH_A + s * SH_RAGS + t * 128 * RAG
            return a[off:off + 128 * RAG].rearrange("(p f) -> p f", f=RAG)

        def agout_mtile(buf, shard, s, m):
            """[128, 512] node m-tile of support s from gathered buf."""
            a = buf.opt()
            q, t = m // NT, m % NT
            off = q * shard + s * SH_MAIN + t * 128 * HALF
            return a[off:off + 128 * HALF].rearrange("(p f) -> p f", f=HALF)

        def agout_ragtile(s, m):
            a = agout1B.opt()
            q, t = m // NT, m % NT
            off = q * SH_B + SH_A + s * SH_RAGS + t * 128 * RAG
            return a[off:off + 128 * RAG].rearrange("(p f) -> p f", f=RAG)

        def agoutC_cols(h, m):
            """[512 bc, 128 nodes] slice for transpose-read of cand."""
            a = agoutC[h].opt()
            q, t = m // NT, m % NT
            v = a[q * SH_C:(q + 1) * SH_C].rearrange("(r c) -> r c", c=NOWN)
            return v[:, t * 128:(t + 1) * 128]

        # ================= hop-1 sweep (one column half) =================
        def emit_h1_half(pid, mov_load, agin_buf):
            """Y1[own, half cols] for both supports; write agin_buf main."""
            ps = {}
            for s in range(2):
                for n in range(NT):
                    ps[(s, n)] = psum.tile([128, HALF], F32,
                                           name=f"ps{pid}{s}{n}",
                                           tag="acc", bufs=8)
            for m in range(MT):
                mv = sb_mov.tile([128, HALF], BF16, name=f"mv{pid}_{m}",
                                 tag="mov", bufs=6)
                mov_load(mv, m)
                for s in range(2):
                    for n in range(NT):
                        nc.tensor.matmul(
                            ps[(s, n)][:],
                            T_tile(s, m)[:, n * 128:(n + 1) * 128],
                            mv[:], start=(m == 0), stop=(m == MT - 1))
            for s in range(2):
                for n in range(NT):
                    ex = sb_ex.tile([128, HALF], BF16, name=f"ex{pid}{s}{n}",
                                    tag="ex", bufs=6)
                    nc.vector.tensor_copy(ex[:], ps[(s, n)][:])
                    nc.sync.dma_start(agin_main(agin_buf, s, n), ex[:])

        # ================= hop-2 (one chunk, one support) ================
        def emit_h2_chunk(pid, s, agout_buf, shard, jbase, x0_tiles, y2_dst):
            """Y2^T[bc chunk, own] = 2*(A_s@Y1)^T - x0^T; keep in SBUF."""
            ps = [psum.tile([128, NOWN], F32, name=f"ph2{pid}{jj}",
                            tag="acc", bufs=8) for jj in range(4)]
            for m in range(MT):
                st = sb_mov.tile([128, HALF], BF16, name=f"st{pid}_{m}",
                                 tag="mov", bufs=6)
                nc.scalar.dma_start(st[:], agout_mtile(agout_buf, shard, s, m))
                for jj in range(4):
                    nc.tensor.matmul(
                        ps[jj][:], st[:, jj * 128:(jj + 1) * 128],
                        T_tile(s, m), start=(m == 0), stop=False)
            for jj in range(4):
                nc.tensor.matmul(ps[jj][:], negI[:], x0_tiles[jbase + jj][:],
                                 start=False, stop=True)
                y2t = sb_y2.tile([128, NOWN], BF16, name=f"y2{pid}{jj}",
                                 tag="y2", bufs=16)
                nc.scalar.mul(y2t[:], ps[jj][:], 2.0)
                y2_dst[(s, jbase + jj)] = y2t

        # ================= W stage (one batch) =================
        def emit_w_gate(b, y1f, y2d):
            g, r2 = b // 4, (b % 4) * 32
            p, o = b // 2, (b % 2) * 64
            zr = psum.tile([128, NOWN], F32, name=f"zr{b}", tag="acc",
                           bufs=8)
            blocks = [stateT[p], y1f[(0, p)], y1f[(1, p)],
                      y2d[(0, p)], y2d[(1, p)]]
            for j in range(5):
                nc.tensor.matmul(zr[:], wg_m[o:o + 64, j, :],
                                 blocks[j][o:o + 64, :],
                                 start=(j == 0), stop=False)
            nc.tensor.matmul(zr[:], wg_x[r2:r2 + 10, :],
                             xall[g][r2:r2 + 10, :],
                             start=False, stop=True, skip_group_check=True,
                             tile_position=(r2, 0))
            ztmp = sb_sm.tile([128, NOWN], BF16, name=f"zt{b}", tag="ztmp",
                              bufs=3)
            nc.scalar.activation(ztmp[o:o + 64, :], zr[0:64, :], AF.Sigmoid,
                                 bias=bg_t[0:64])
            nc.scalar.activation(rt[p][o:o + 64, :], zr[64:128, :],
                                 AF.Sigmoid, bias=bg_t[64:128])
            nc.vector.tensor_mul(zs[p][o:o + 64, :], ztmp[o:o + 64, :],
                                 stateT[p][o:o + 64, :])
            nc.sync.dma_start(
                aginC[b // 8].opt()[(b % 8) * 64 * NOWN:
                                    ((b % 8) + 1) * 64 * NOWN]
                .rearrange("(r c) -> r c", c=NOWN),
                zs[p][o:o + 64, :])

        def emit_w_update(b, y1f, y2d):
            g, r2 = b // 4, (b % 4) * 32
            p, o = b // 2, (b % 2) * 64
            hcp = psum.tile([H, NOWN], F32, name=f"hc{b}", tag="acc",
                            bufs=8)
            blocks = [zs[p], y1f[(0, p)], y1f[(1, p)],
                      y2d[(0, p)], y2d[(1, p)]]
            for j in range(5):
                nc.tensor.matmul(hcp[:], wu_m[o:o + 64, j, :],
                                 blocks[j][o:o + 64, :],
                                 start=(j == 0), stop=False)
            nc.tensor.matmul(hcp[:], wu_x[r2:r2 + 10, :],
                             xall[g][r2:r2 + 10, :],
                             start=False, stop=True, skip_group_check=True,
                             tile_position=(r2, 0))
            hc = sb_sm.tile([128, NOWN], BF16, name=f"hcs{b}", tag="hc",
                            bufs=3)
            nc.scalar.activation(hc[o:o + 64, :], hcp[:], AF.Tanh,
                                 bias=bu_t[:])
            t1 = sb_sm.tile([128, NOWN], BF16, name=f"t1{b}", tag="t1",
                            bufs=3)
            nc.vector.tensor_sub(t1[o:o + 64, :], stateT[p][o:o + 64, :],
                                 hc[o:o + 64, :])
            nc.vector.tensor_mul(t1[o:o + 64, :], rt[p][o:o + 64, :],
                                 t1[o:o + 64, :])
            ot = sb_sm.tile([H, NOWN], F32, name=f"ot{b}", tag="ot", bufs=3)
            nc.vector.tensor_add(ot[:], hc[o:o + 64, :], t1[o:o + 64, :])
            nc.sync.dma_start(d["outT"].ap()[b], ot[:])

        def ag(in_t, out_t):
            nc.gpsimd.collective_compute(
                "AllGather", mybir.AluOpType.bypass, replica_groups=GROUP,
                ins=[in_t.opt()], outs=[out_t.opt()])

        # ======================= GCN 1 =======================
        emit_h1_half(
            "g1h0",
            lambda mv, m: nc.sync.dma_start(
                mv[:], d["xs_main"].ap()[m * 128:(m + 1) * 128, 0:HALF]),
            agin1A)
        ag(agin1A, agout1A)
        emit_h1_half(
            "g1h1",
            lambda mv, m: nc.sync.dma_start(
                mv[:], d["xs_main"].ap()[m * 128:(m + 1) * 128, HALF:MAIN]),
            agin1B)

        # ragged x hop-1 (shared across GCNs): Y1x^T = (A_s @ x)^T
        psx = [psum.tile([RAG, NOWN], F32, name=f"psx{s}", tag="acc",
                         bufs=8) for s in range(2)]
        for m in range(MT):
            mvr = sb_mov.tile([128, RAG], BF16, name=f"mvr{m}", tag="movr",
                              bufs=6)
            nc.sync.dma_start(mvr[:],
                              d["xs_rag"].ap()[m * 128:(m + 1) * 128, :])
            for s in range(2):
                nc.tensor.matmul(psx[s][:], mvr[:], T_tile(s, m),
                                 start=(m == 0), stop=(m == MT - 1))
        for s in range(2):
            o = 32 + 32 * s
            nc.vector.tensor_copy(S1[o:o + 32, :], psx[s][:])
            for t in range(NT):
                tp = psum.tile([128, RAG], BF16, name=f"tpx{s}{t}",
                               tag="acc", bufs=8)
                nc.tensor.transpose(tp[:],
                                    S1[o:o + 32, t * 128:(t + 1) * 128],
                                    identb[o:o + 32, o:o + 32])
                rnm = sb_sm.tile([128, RAG], BF16, name=f"rnm{s}{t}",
                                 tag="rnm", bufs=4)
                nc.vector.tensor_copy(rnm[:], tp[:])
                nc.sync.dma_start(agin_rag(s, t), rnm[:])
        ag(agin1B, agout1B)

        # feature-major Y1 blocks for W-gate via transpose reads (local)
        y1f1 = {}
        for s in range(2):
            for p in range(8):
                t = sb_y1.tile([128, NOWN], BF16, name=f"y1f1{s}{p}",
                               tag="y1f", bufs=16)
                src = agin_view(agin1A if p < 4 else agin1B, s)
                nc.scalar.dma_start(
                    t[:], src[:, (p % 4) * 128:(p % 4 + 1) * 128],
                    transpose=True)
                y1f1[(s, p)] = t

        # hop-2 chunk A (cols 0..511 = batches 0..7)
        y2d1 = {}
        emit_h2_chunk("g1s0A", 0, agout1A, SH_A, 0, stateT, y2d1)
        emit_h2_chunk("g1s1A", 1, agout1A, SH_A, 0, stateT, y2d1)

        # x hop-2 (shared): Y2x^T = 2*(A_s @ y1x)^T - x^T
        psx2 = [psum.tile([RAG, NOWN], F32, name=f"psx2{s}", tag="acc",
                          bufs=8) for s in range(2)]
        for m in range(MT):
            for s in range(2):
                mvr = sb_mov.tile([128, RAG], BF16, name=f"mvr2{s}{m}",
                                  tag="movr", bufs=6)
                nc.scalar.dma_start(mvr[:], agout_ragtile(s, m))
                nc.tensor.matmul(psx2[s][:], mvr[:], T_tile(s, m),
                                 start=(m == 0), stop=False)
        for s in range(2):
            nc.tensor.matmul(psx2[s][:], negI[0:RAG, 0:RAG], S1[0:RAG, :],
                             start=False, stop=True)
            nc.scalar.mul(S2[32 * s:32 * s + 32, :], psx2[s][:], 2.0)
        # assemble XALL group tiles via PE permutation matmuls
        for g in range(4):
            psa = psum.tile([128, NOWN], F32, name=f"psa{g}", tag="acc",
                            bufs=8)
            nc.tensor.matmul(psa[:], P1[:, g, :], S1[:], start=True,
                             stop=False)
            nc.tensor.matmul(psa[:], P2[:, g, :], S2[:], start=False,
                             stop=True)
            nc.vector.tensor_copy(xall[g][:], psa[:])

        # W-gate batches 0..7 (needs chunk A), then cand AG half 0
        for b in range(8):
            emit_w_gate(b, y1f1, y2d1)
        ag(aginC[0], agoutC[0])

        # hop-2 chunk B (cols 512..1023 = batches 8..15)
        emit_h2_chunk("g1s0B", 0, agout1B, SH_B, 4, stateT, y2d1)
        emit_h2_chunk("g1s1B", 1, agout1B, SH_B, 4, stateT, y2d1)
        for b in range(8, 16):
            emit_w_gate(b, y1f1, y2d1)
        ag(aginC[1], agoutC[1])

        # ======================= GCN 2 =======================
        emit_h1_half(
            "g2h0",
            lambda mv, m: nc.scalar.dma_start(mv[:], agoutC_cols(0, m),
                                              transpose=True),
            agin2A)
        ag(agin2A, agout2A)
        emit_h1_half(
            "g2h1",
            lambda mv, m: nc.scalar.dma_start(mv[:], agoutC_cols(1, m),
                                              transpose=True),
            agin2B)
        ag(agin2B, agout2B)

        y1f2 = {}
        for s in range(2):
            for p in range(8):
                t = sb_y1.tile([128, NOWN], BF16, name=f"y1f2{s}{p}",
                               tag="y1f", bufs=16)
                src = agin_view(agin2A if p < 4 else agin2B, s)
                nc.scalar.dma_start(
                    t[:], src[:, (p % 4) * 128:(p % 4 + 1) * 128],
                    transpose=True)
                y1f2[(s, p)] = t

        y2d2 = {}
        emit_h2_chunk("g2s0A", 0, agout2A, SH_A, 0, zs, y2d2)
        emit_h2_chunk("g2s1A", 1, agout2A, SH_A, 0, zs, y2d2)
        for b in range(8):
            emit_w_update(b, y1f2, y2d2)
        emit_h2_chunk("g2s0B", 0, agout2B, SH_A, 4, zs, y2d2)
        emit_h2_chunk("g2s1B", 1, agout2B, SH_A, 4, zs, y2d2)
        for b in range(8, 16):
            emit_w_update(b, y1f2, y2d2)


def prepare_in_maps(x, state, support0, support1, W_gate, b_gate,
                    W_update, b_update):
    import ml_dtypes
    bf16 = ml_dtypes.bfloat16

    x = np.asarray(x, dtype=np.float32)
    state = np.asarray(state, dtype=np.float32)
    xs_main = np.ascontiguousarray(
        state.transpose(1, 0, 2).reshape(N, MAIN)).astype(bf16)
    xs_rag = np.ascontiguousarray(
        x.transpose(1, 0, 2).reshape(N, RAG)).astype(bf16)

    def pack_w(W, cols):
        # reference feats order: [xs, Y1s0, Y2s0, Y1s1, Y2s1] (66 rows each,
        # [x(2); state(64)]).  Our K=64 block order: xs, Y1s0, Y1s1, Y2s0,
        # Y2s1; x rows go to the XALL block in the same order.
        order = [0, 1, 3, 2, 4]
        Wm = np.zeros((5, 128, cols), dtype=np.float32)
        Wx10 = np.zeros((10, cols), dtype=np.float32)
        for i, j in enumerate(order):
            blk = W[j * 66:(j + 1) * 66]
            Wm[i, 0:64] = blk[2:66]
            Wm[i, 64:128] = blk[2:66]
            Wx10[2 * i:2 * i + 2] = blk[0:2]
        Wx = np.zeros((128, cols), dtype=np.float32)
        for base in (0, 32, 64, 96):
            Wx[base:base + 10] = Wx10
        return Wm.astype(bf16), Wx.astype(bf16)

    Wg_main, Wg_x = pack_w(np.asarray(W_gate, dtype=np.float32), 128)
    Wu_main, Wu_x = pack_w(np.asarray(W_update, dtype=np.float32), H)
    bg = np.ascontiguousarray(b_gate, dtype=np.float32).reshape(2 * H, 1)
    bu = np.ascontiguousarray(b_update, dtype=np.float32).reshape(H, 1)
    negI = (-0.5 * np.eye(128)).astype(bf16)
    identb = np.eye(128).astype(bf16)

    # XALL selection: group g, local batch lb (b = 4g+lb), dest rows
    # lb*32 + [x(2), y1x_s0(2), y1x_s1(2), y2x_s0(2), y2x_s1(2)]
    P1 = np.zeros((4, 96, 128), dtype=np.float32)
    P2 = np.zeros((4, 64, 128), dtype=np.float32)
    for g in range(4):
        for lb in range(4):
            b = 4 * g + lb
            for k in range(2):
                P1[g, 2 * b + k, lb * 32 + k] = 1.0          # x
                P1[g, 32 + 2 * b + k, lb * 32 + 2 + k] = 1.0  # y1x s0
                P1[g, 64 + 2 * b + k, lb * 32 + 4 + k] = 1.0  # y1x s1
                P2[g, 2 * b + k, lb * 32 + 6 + k] = 1.0       # y2x s0
                P2[g, 32 + 2 * b + k, lb * 32 + 8 + k] = 1.0  # y2x s1
    P1 = P1.astype(bf16)
    P2 = P2.astype(bf16)

    in_maps = []
    for r in range(NCORES):
        n0 = r * NOWN
        stT = np.ascontiguousarray(
            state[:, n0:n0 + NOWN, :].transpose(0, 2, 1)
            .reshape(8, 128, NOWN)).astype(bf16)
        xTc = np.ascontiguousarray(
            x[:, n0:n0 + NOWN, :].transpose(0, 2, 1)
            .reshape(RAG, NOWN)).astype(bf16)
        in_maps.append({
            "Ts": np.ascontiguousarray(
                np.stack([support0[n0:n0 + NOWN, :].T,
                          support1[n0:n0 + NOWN, :].T])).astype(bf16),
            "xs_main": xs_main,
            "xs_rag": xs_rag,
            "stateT": stT,
            "xT": xTc,
            "P1": P1, "P2": P2,
            "Wg_main": Wg_main, "Wg_x": Wg_x,
            "Wu_main": Wu_main, "Wu_x": Wu_x,
            "bg": bg, "bu": bu, "negI": negI, "identb": identb,
        })
    return in_maps


def assemble_output(results):
    out = np.empty((B, N, H), dtype=np.float32)
    for r in range(NCORES):
        n0 = r * NOWN
        out[:, n0:n0 + NOWN, :] = results[r]["outT"].transpose(0, 2, 1)
    return out


def get_nc():
    if "nc" not in _NC_CACHE:
        _NC_CACHE["nc"] = build_nc()
    return _NC_CACHE["nc"]


def kernel(x, state, support0, support1, W_gate, b_gate, W_update, b_update):
    nc = get_nc()
    in_maps = prepare_in_maps(x, state, support0, support1,
                              W_gate, b_gate, W_update, b_update)
    prev = os.environ.get("BASS_NEVER_TRACE")
    os.environ["BASS_NEVER_TRACE"] = "1"
    try:
        res = run_bass_kernel_spmd(nc, in_maps, list(range(NCORES)),
                                   trace=False)
    finally:
        if prev is None:
            os.environ.pop("BASS_NEVER_TRACE", None)
        else:
            os.environ["BASS_NEVER_TRACE"] = prev
    return assemble_output(res.results)


# revision 22
# speedup vs baseline: 1.6932x; 1.1209x over previous
"""DCGRU cell (nn_DCGRUCell) Trainium2 Bass kernel, 8 NeuronCores — v3.

Sharding: node dim N=4096 split 8 ways (512/core); supports resident in
SBUF as fp8_e4m3 scaled by 4096 (values in [0,1]).  Diffusion matmuls
run fp8 (hop products staged/AllGathered as fp8 scaled by 32); the
undiffused feature block, W stage and gate math run bf16/fp32, so the
large-magnitude path keeps precision.  Scales are powers of two and are
folded into copy-out activations and host-side W rows (exact).

Node-major moving layout splits columns [B*H=1024 state | B*2=32 x];
the x-feature diffusion is identical for both GCNs and computed once.
Hop-1 outputs AllGather in two column halves so each collective hides
under the next compute phase.  Hop-1 results are double-written (fp8
AG input + bf16 local copy) so feature-major W blocks transpose-read
the local copy without serializing on the collective.  Hop-2 outputs
and all W-stage operands stay in SBUF.

kernel(**inputs) takes FULL inputs, returns FULL [16,4096,64] fp32.
"""
import os
import numpy as np

import concourse.mybir as mybir
import concourse.tile as tile
from concourse import bacc
from concourse.bass_utils import run_bass_kernel_spmd

F32 = mybir.dt.float32
BF16 = mybir.dt.bfloat16
FP8 = mybir.dt.float8e4
AF = mybir.ActivationFunctionType

NCORES = 8
B, N, H, DIN = 16, 4096, 64, 2
NOWN = N // NCORES          # 512 rows per core
NT = NOWN // 128            # 4 n-tiles per core
MT = N // 128               # 32 m-tiles (contraction)
MAIN = B * H                # 1024 state columns, batch-major
RAG = B * DIN               # 32 x columns, batch-major
HALF = 512                  # column half for AG chunking
SH_MAIN = NOWN * HALF       # elems of one support's half in a shard
SH_A = 2 * SH_MAIN          # chunk-A shard elems (both supports)
SH_RAGS = NOWN * RAG        # one support's ragged elems
SH_B = SH_A + 2 * SH_RAGS   # chunk-B shard elems
SH_C = NOWN * HALF          # cand chunk shard elems
GROUP = [list(range(NCORES))]

_NC_CACHE = {}


def build_nc():
    nc = bacc.Bacc("TRN2", target_bir_lowering=False, debug=False,
                   num_devices=NCORES)
    d = {}
    d["Ts"] = nc.dram_tensor("Ts", [2, N, NOWN], FP8, kind="ExternalInput")
    d["xs_main"] = nc.dram_tensor("xs_main", [N, MAIN], FP8,
                                  kind="ExternalInput")
    d["xs_rag"] = nc.dram_tensor("xs_rag", [N, RAG], FP8,
                                 kind="ExternalInput")
    d["stateT"] = nc.dram_tensor("stateT", [8, 128, NOWN], BF16,
                                 kind="ExternalInput")
    d["stateTf"] = nc.dram_tensor("stateTf", [8, 128, NOWN], F32,
                                  kind="ExternalInput")
    d["xT"] = nc.dram_tensor("xT", [RAG, NOWN], BF16, kind="ExternalInput")
    d["P1"] = nc.dram_tensor("P1", [4, 96, 128], BF16, kind="ExternalInput")
    d["P2"] = nc.dram_tensor("P2", [4, 2, RAG, 128], BF16,
                             kind="ExternalInput")
    d["Wg_main"] = nc.dram_tensor("Wg_main", [5, 128, 128], BF16,
                                  kind="ExternalInput")
    d["Wg_x"] = nc.dram_tensor("Wg_x", [128, 128], BF16,
                               kind="ExternalInput")
    d["Wu_main"] = nc.dram_tensor("Wu_main", [5, 128, H], BF16,
                                  kind="ExternalInput")
    d["Wu_x"] = nc.dram_tensor("Wu_x", [128, H], BF16, kind="ExternalInput")
    d["bg"] = nc.dram_tensor("bg", [2 * H, 1], F32, kind="ExternalInput")
    d["bu"] = nc.dram_tensor("bu", [H, 1], F32, kind="ExternalInput")
    d["identb"] = nc.dram_tensor("identb", [128, 128], BF16,
                                 kind="ExternalInput")
    d["outT"] = nc.dram_tensor("outT", [B, H, NOWN], F32,
                               kind="ExternalOutput")
    with tile.TileContext(nc) as tc:
        _emit(nc, tc, d)
    nc.compile()
    return nc


def _emit(nc, tc, d):
    import contextlib
    stack = contextlib.ExitStack()
    with stack:
        const = stack.enter_context(tc.tile_pool(name="const", bufs=1))
        res = stack.enter_context(tc.tile_pool(name="res", bufs=1))
        sb_mov = stack.enter_context(tc.tile_pool(name="mov", bufs=1))
        sb_ex = stack.enter_context(tc.tile_pool(name="ex", bufs=1))
        sb_y1 = stack.enter_context(tc.tile_pool(name="y1f", bufs=1))
        sb_y2 = stack.enter_context(tc.tile_pool(name="y2", bufs=1))
        sb_sm = stack.enter_context(tc.tile_pool(name="small", bufs=1))
        dram = stack.enter_context(
            tc.tile_pool(name="dram", bufs=1, space="DRAM"))
        psum = stack.enter_context(
            tc.tile_pool(name="psum", bufs=1, space="PSUM"))

        # ---- early loads: supports on the scalar HWDGE queue so the
        # sync queue serves the first sweep's moving tiles immediately
        CH = 4
        NCH = MT // CH
        Tch = {}
        for s in range(2):
            for k in range(NCH):
                Tch[(s, k)] = const.tile([128, CH, NOWN], FP8,
                                         name=f"T{s}_{k}")
        for k in range(NCH):
            for s in range(2):
                ts = d["Ts"].ap()[s].rearrange("(t p) n -> p t n", p=128)
                nc.scalar.dma_start(Tch[(s, k)][:],
                                    ts[:, k * CH:(k + 1) * CH, :])

        identb = const.tile([128, 128], BF16)
        nc.sync.dma_start(identb[:], d["identb"].ap())
        xr_all = const.tile([128, MT, RAG], FP8)
        nc.sync.dma_start(
            xr_all[:],
            d["xs_rag"].ap().rearrange("(m p) c -> p m c", p=128))

        def T_tile(s, m):
            return Tch[(s, m // CH)][:, m % CH, :]

        # ---- resident tiles (loads deferred; emitted after h1 below)
        S1 = res.tile([96, NOWN], BF16, name="S1")
        y2x = [res.tile([RAG, NOWN], BF16, name=f"y2x{s}") for s in range(2)]
        P1 = const.tile([96, 4, 128], BF16)
        P2 = const.tile([RAG, 4, 2, 128], BF16)
        stateT = [res.tile([128, NOWN], BF16, name=f"stT{p}")
                  for p in range(8)]
        stateTf = [res.tile([128, NOWN], F32, name=f"stTf{p}")
                   for p in range(8)]
        xall = [res.tile([128, NOWN], BF16, name=f"xall{g}")
                for g in range(4)]
        wg_m = const.tile([128, 5, 128], BF16)
        wu_m = const.tile([128, 5, H], BF16)
        wg_x = const.tile([128, 128], BF16)
        wu_x = const.tile([128, H], BF16)
        bg_t = const.tile([2 * H, 1], F32)
        bu_t = const.tile([H, 1], F32)
        rt = [res.tile([128, NOWN], F32, name=f"rt{p}") for p in range(8)]
        zs = [res.tile([128, NOWN], BF16, name=f"zs{p}") for p in range(8)]

        def emit_resident_loads():
            nc.sync.dma_start(S1[0:RAG, :], d["xT"].ap())
            for p in range(8):
                nc.sync.dma_start(stateT[p][:], d["stateT"].ap()[p])
                nc.sync.dma_start(stateTf[p][:], d["stateTf"].ap()[p])
            for j in range(5):
                nc.sync.dma_start(wg_m[:, j, :], d["Wg_main"].ap()[j])
                nc.sync.dma_start(wu_m[:, j, :], d["Wu_main"].ap()[j])
            nc.sync.dma_start(wg_x[:], d["Wg_x"].ap())
            nc.sync.dma_start(wu_x[:], d["Wu_x"].ap())
            nc.sync.dma_start(bg_t[:], d["bg"].ap())
            nc.sync.dma_start(bu_t[:], d["bu"].ap())
            for g in range(4):
                nc.sync.dma_start(P1[:, g, :], d["P1"].ap()[g])
                for s in range(2):
                    nc.sync.dma_start(P2[:, g, s, :], d["P2"].ap()[g, s])

        # ---------------- DRAM staging ----------------
        agin1A = dram.tile([SH_A], FP8, name="agin1A")
        agin1B = dram.tile([SH_B], FP8, name="agin1B")
        agout1A = dram.tile([NCORES * SH_A], FP8, name="agout1A",
                            addr_space="Shared")
        agout1B = dram.tile([NCORES * SH_B], FP8, name="agout1B",
                            addr_space="Shared")
        aginC = [dram.tile([SH_C], BF16, name=f"aginC{h}") for h in range(2)]
        agoutC = [dram.tile([NCORES * SH_C], BF16, name=f"agoutC{h}",
                            addr_space="Shared") for h in range(2)]
        agin2A = dram.tile([SH_A], FP8, name="agin2A")
        agin2B = dram.tile([SH_A], FP8, name="agin2B")
        agout2A = dram.tile([NCORES * SH_A], FP8, name="agout2A",
                            addr_space="Shared")
        agout2B = dram.tile([NCORES * SH_A], FP8, name="agout2B",
                            addr_space="Shared")
        y1loc = [dram.tile([2 * NOWN * MAIN], BF16, name=f"y1loc{i}")
                 for i in range(2)]

        def agin_main(buf, s, t):
            a = buf.opt()
            off = s * SH_MAIN + t * 128 * HALF
            return a[off:off + 128 * HALF].rearrange("(p f) -> p f", f=HALF)

        def agin_rag(s, t):
            a = agin1B.opt()
            off = SH_A + s * SH_RAGS + t * 128 * RAG
            return a[off:off + 128 * RAG].rearrange("(p f) -> p f", f=RAG)

        def agout_mtile(buf, shard, s, m):
            a = buf.opt()
            q, t = m // NT, m % NT
            off = q * shard + s * SH_MAIN + t * 128 * HALF
            return a[off:off + 128 * HALF].rearrange("(p f) -> p f", f=HALF)

        def agout_rag_block(s, q):
            a = agout1B.opt()
            off = q * SH_B + SH_A + s * SH_RAGS
            return a[off:off + SH_RAGS].rearrange("(t p c) -> p t c", p=128,
                                                  c=RAG)

        def agoutC_cols(h, m):
            a = agoutC[h].opt()
            q, t = m // NT, m % NT
            v = a[q * SH_C:(q + 1) * SH_C].rearrange("(r c) -> r c", c=NOWN)
            return v[:, t * 128:(t + 1) * 128]

        def y1loc_tile(i, s, t, h):
            """[128, 512] write view: own node tile t, col half h."""
            a = y1loc[i].opt()
            off = s * NOWN * MAIN + t * 128 * MAIN
            v = a[off:off + 128 * MAIN].rearrange("(p f) -> p f", f=MAIN)
            return v[:, h * HALF:(h + 1) * HALF]

        def y1loc_cols(i, s, p):
            """[512, 128] read view for transpose: bc cols p*128."""
            a = y1loc[i].opt()
            v = a[s * NOWN * MAIN:(s + 1) * NOWN * MAIN].rearrange(
                "(n f) -> n f", f=MAIN)
            return v[:, p * 128:(p + 1) * 128]

        # ================= hop-1 sweep (one column half) =================
        def emit_h1_half(pid, mov_load, agin_buf, gi, h):
            """Y1[own, half cols] both supports; psum holds 4096*Y1.
            Writes fp8 32*Y1 to agin_buf and bf16 32*Y1 to y1loc."""
            ps = {}
            for s in range(2):
                for n in range(NT):
                    ps[(s, n)] = psum.tile([128, HALF], F32,
                                           name=f"ps{pid}{s}{n}",
                                           tag="acc", bufs=8)
            for m in range(MT):
                mv = sb_mov.tile([128, HALF],
                                 BF16 if gi == 1 else FP8,
                                 name=f"mv{pid}_{m}", tag="mov", bufs=6)
                mov_load(mv, m)
                for s in range(2):
                    for n in range(NT):
                        nc.tensor.matmul(
                            ps[(s, n)][:],
                            T_tile(s, m)[:, n * 128:(n + 1) * 128],
                            mv[:], start=(m == 0), stop=(m == MT - 1))
            for s in range(2):
                for n in range(NT):
                    ex8 = sb_ex.tile([128, HALF], FP8, name=f"e8{pid}{s}{n}",
                                     tag="ex8", bufs=6)
                    nc.scalar.mul(ex8[:], ps[(s, n)][:], 2.0 ** -7)
                    nc.sync.dma_start(agin_main(agin_buf, s, n), ex8[:])
                    exb = sb_ex.tile([128, HALF], BF16, name=f"eb{pid}{s}{n}",
                                     tag="exb", bufs=6)
                    nc.vector.tensor_scalar_mul(exb[:], ps[(s, n)][:],
                                                2.0 ** -7)
                    nc.sync.dma_start(y1loc_tile(gi, s, n, h), exb[:])

        # ================= hop-2 (one chunk, one support) ================
        def emit_h2_chunk(pid, s, agout_buf, shard, jbase, x0_tiles, y2_dst):
            """Y2^T[bc chunk, own] = 2*(A_s@Y1)^T - x0^T; stays in SBUF.
            Stationary fp8 is 32*Y1, T is 4096*A -> psum = 2^17*(A@Y1)."""
            ps = [psum.tile([128, NOWN], F32, name=f"ph2{pid}{jj}",
                            tag="acc", bufs=8) for jj in range(4)]
            for m in range(MT):
                st = sb_mov.tile([128, HALF], FP8, name=f"st{pid}_{m}",
                                 tag="st", bufs=6)
                nc.scalar.dma_start(st[:], agout_mtile(agout_buf, shard, s, m))
                for jj in range(4):
                    nc.tensor.matmul(
                        ps[jj][:], st[:, jj * 128:(jj + 1) * 128],
                        T_tile(s, m), start=(m == 0), stop=(m == MT - 1))
            for jj in range(4):
                y2t = sb_y2.tile([128, NOWN], BF16, name=f"y2{pid}{jj}",
                                 tag="y2", bufs=16)
                nc.scalar.mul(y2t[:], ps[jj][:], 2.0 ** -16)
                nc.vector.tensor_sub(y2t[:], y2t[:], x0_tiles[jbase + jj][:])
                y2_dst[(s, jbase + jj)] = y2t

        # ================= W stage (one batch) =================
        def emit_w_gate(b, y1f, y2d):
            g, r2 = b // 4, (b % 4) * 32
            p, o = b // 2, (b % 2) * 64
            zr = psum.tile([128, NOWN], F32, name=f"zr{b}", tag="acc",
                           bufs=8)
            blocks = [stateT[p], y1f[(0, p)], y1f[(1, p)],
                      y2d[(0, p)], y2d[(1, p)]]
            for j in range(5):
                nc.tensor.matmul(zr[:], wg_m[o:o + 64, j, :],
                                 blocks[j][o:o + 64, :],
                                 start=(j == 0), stop=False)
            nc.tensor.matmul(zr[:], wg_x[r2:r2 + 10, :],
                             xall[g][r2:r2 + 10, :],
                             start=False, stop=True, skip_group_check=True,
                             tile_position=(r2, 0))
            ztmp = sb_sm.tile([128, NOWN], BF16, name=f"zt{b}", tag="ztmp",
                              bufs=2)
            nc.scalar.activation(ztmp[o:o + 64, :], zr[0:64, :], AF.Sigmoid,
                                 bias=bg_t[0:64])
            nc.scalar.activation(rt[p][o:o + 64, :], zr[64:128, :],
                                 AF.Sigmoid, bias=bg_t[64:128])
            nc.vector.tensor_mul(zs[p][o:o + 64, :], ztmp[o:o + 64, :],
                                 stateT[p][o:o + 64, :])
            nc.sync.dma_start(
                aginC[b // 8].opt()[(b % 8) * 64 * NOWN:
                                    ((b % 8) + 1) * 64 * NOWN]
                .rearrange("(r c) -> r c", c=NOWN),
                zs[p][o:o + 64, :])

        def emit_w_update(b, y1f, y2d):
            g, r2 = b // 4, (b % 4) * 32
            p, o = b // 2, (b % 2) * 64
            hcp = psum.tile([H, NOWN], F32, name=f"hc{b}", tag="acc",
                            bufs=8)
            blocks = [zs[p], y1f[(0, p)], y1f[(1, p)],
                      y2d[(0, p)], y2d[(1, p)]]
            for j in range(5):
                nc.tensor.matmul(hcp[:], wu_m[o:o + 64, j, :],
                                 blocks[j][o:o + 64, :],
                                 start=(j == 0), stop=False)
            nc.tensor.matmul(hcp[:], wu_x[r2:r2 + 10, :],
                             xall[g][r2:r2 + 10, :],
                             start=False, stop=True, skip_group_check=True,
                             tile_position=(r2, 0))
            hc = sb_sm.tile([128, NOWN], F32, name=f"hcs{b}", tag="hc",
                            bufs=2)
            nc.scalar.activation(hc[o:o + 64, :], hcp[:], AF.Tanh,
                                 bias=bu_t[:])
            t1 = sb_sm.tile([128, NOWN], F32, name=f"t1{b}", tag="t1",
                            bufs=2)
            nc.vector.tensor_sub(t1[o:o + 64, :], stateTf[p][o:o + 64, :],
                                 hc[o:o + 64, :])
            nc.vector.tensor_mul(t1[o:o + 64, :], rt[p][o:o + 64, :],
                                 t1[o:o + 64, :])
            ot = sb_sm.tile([H, NOWN], F32, name=f"ot{b}", tag="ot", bufs=3)
            nc.vector.tensor_add(ot[:], hc[o:o + 64, :], t1[o:o + 64, :])
            nc.sync.dma_start(d["outT"].ap()[b], ot[:])

        def ag(in_t, out_t):
            nc.gpsimd.collective_compute(
                "AllGather", mybir.AluOpType.bypass, replica_groups=GROUP,
                ins=[in_t.opt()], outs=[out_t.opt()])

        # ======================= GCN 1 =======================
        emit_h1_half(
            "g1h0",
            lambda mv, m: nc.sync.dma_start(
                mv[:], d["xs_main"].ap()[m * 128:(m + 1) * 128, 0:HALF]),
            agin1A, 0, 0)
        ag(agin1A, agout1A)

        # ragged x hop-1 (shared across GCNs): psum = 4096*(A_s @ x)^T
        psx = [psum.tile([RAG, NOWN], F32, name=f"psx{s}", tag="acc",
                         bufs=8) for s in range(2)]
        for m in range(MT):
            for s in range(2):
                nc.tensor.matmul(psx[s][:], xr_all[:, m, :], T_tile(s, m),
                                 start=(m == 0), stop=(m == MT - 1))
        for s in range(2):
            o = 32 + 32 * s
            nc.scalar.mul(S1[o:o + 32, :], psx[s][:], 2.0 ** -12)
            for t in range(NT):
                tp = psum.tile([128, RAG], BF16, name=f"tpx{s}{t}",
                               tag="acc", bufs=8)
                nc.tensor.transpose(tp[:],
                                    S1[o:o + 32, t * 128:(t + 1) * 128],
                                    identb[o:o + 32, o:o + 32])
                rnm = sb_sm.tile([128, RAG], FP8, name=f"rnm{s}{t}",
                                 tag="rnm", bufs=4)
                nc.scalar.mul(rnm[:], tp[:], 32.0)
                nc.sync.dma_start(agin_rag(s, t), rnm[:])

        emit_h1_half(
            "g1h1",
            lambda mv, m: nc.sync.dma_start(
                mv[:], d["xs_main"].ap()[m * 128:(m + 1) * 128, HALF:MAIN]),
            agin1B, 0, 1)
        ag(agin1B, agout1B)

        emit_resident_loads()

        # feature-major Y1 blocks for W-gate from the local bf16 copy
        y1f1 = {}
        for s in range(2):
            for p in range(8):
                t = sb_y1.tile([128, NOWN], BF16, name=f"y1f1{s}{p}",
                               tag="y1f", bufs=16)
                nc.scalar.dma_start(t[:], y1loc_cols(0, s, p),
                                    transpose=True)
                y1f1[(s, p)] = t

        # hop-2 chunk A (cols 0..511 = batches 0..7)
        y2d1 = {}
        emit_h2_chunk("g1s0A", 0, agout1A, SH_A, 0, stateT, y2d1)
        emit_h2_chunk("g1s1A", 1, agout1A, SH_A, 0, stateT, y2d1)

        # x hop-2 (shared): y2x = 2*(A_s @ y1x)^T - x^T
        xr2 = [const.tile([128, MT, RAG], FP8, name=f"xr2_{s}")
               for s in range(2)]
        for s in range(2):
            for q in range(NCORES):
                nc.sync.dma_start(xr2[s][:, q * NT:(q + 1) * NT, :],
                                  agout_rag_block(s, q))
        psx2 = [psum.tile([RAG, NOWN], F32, name=f"psx2{s}", tag="acc",
                          bufs=8) for s in range(2)]
        for m in range(MT):
            for s in range(2):
                nc.tensor.matmul(psx2[s][:], xr2[s][:, m, :], T_tile(s, m),
                                 start=(m == 0), stop=(m == MT - 1))
        for s in range(2):
            nc.scalar.mul(y2x[s][:], psx2[s][:], 2.0 ** -16)
            nc.vector.tensor_sub(y2x[s][:], y2x[s][:], S1[0:RAG, :])
        # assemble XALL group tiles via PE permutation matmuls
        for g in range(4):
            psa = psum.tile([128, NOWN], F32, name=f"psa{g}", tag="acc",
                            bufs=8)
            nc.tensor.matmul(psa[:], P1[:, g, :], S1[:], start=True,
                             stop=False)
            nc.tensor.matmul(psa[:], P2[:, g, 0, :], y2x[0][:],
                             start=False, stop=False)
            nc.tensor.matmul(psa[:], P2[:, g, 1, :], y2x[1][:],
                             start=False, stop=True)
            nc.vector.tensor_copy(xall[g][:], psa[:])

        # W-gate batches 0..7 (needs chunk A), then cand AG half 0
        for b in range(8):
            emit_w_gate(b, y1f1, y2d1)
        ag(aginC[0], agoutC[0])

        # hop-2 chunk B (cols 512..1023 = batches 8..15)
        emit_h2_chunk("g1s0B", 0, agout1B, SH_B, 4, stateT, y2d1)
        emit_h2_chunk("g1s1B", 1, agout1B, SH_B, 4, stateT, y2d1)
        for b in range(8, 16):
            emit_w_gate(b, y1f1, y2d1)
        ag(aginC[1], agoutC[1])

        # ======================= GCN 2 =======================
        emit_h1_half(
            "g2h0",
            lambda mv, m: nc.scalar.dma_start(mv[:], agoutC_cols(0, m),
                                              transpose=True),
            agin2A, 1, 0)
        ag(agin2A, agout2A)
        emit_h1_half(
            "g2h1",
            lambda mv, m: nc.scalar.dma_start(mv[:], agoutC_cols(1, m),
                                              transpose=True),
            agin2B, 1, 1)
        ag(agin2B, agout2B)

        y1f2 = {}
        for s in range(2):
            for p in range(8):
                t = sb_y1.tile([128, NOWN], BF16, name=f"y1f2{s}{p}",
                               tag="y1f", bufs=16)
                nc.scalar.dma_start(t[:], y1loc_cols(1, s, p),
                                    transpose=True)
                y1f2[(s, p)] = t

        y2d2 = {}
        emit_h2_chunk("g2s0A", 0, agout2A, SH_A, 0, zs, y2d2)
        emit_h2_chunk("g2s1A", 1, agout2A, SH_A, 0, zs, y2d2)
        for b in range(8):
            emit_w_update(b, y1f2, y2d2)
        emit_h2_chunk("g2s0B", 0, agout2B, SH_A, 4, zs, y2d2)
        emit_h2_chunk("g2s1B", 1, agout2B, SH_A, 4, zs, y2d2)
        for b in range(8, 16):
            emit_w_update(b, y1f2, y2d2)


def prepare_in_maps(x, state, support0, support1, W_gate, b_gate,
                    W_update, b_update):
    import ml_dtypes
    bf16 = ml_dtypes.bfloat16
    fp8 = ml_dtypes.float8_e4m3fn

    x = np.asarray(x, dtype=np.float32)
    state = np.asarray(state, dtype=np.float32)
    xs_main = np.ascontiguousarray(
        state.transpose(1, 0, 2).reshape(N, MAIN)).astype(fp8)
    xs_rag = np.ascontiguousarray(
        x.transpose(1, 0, 2).reshape(N, RAG)).astype(fp8)

    def pack_w(W, cols):
        # reference feats order: [xs, Y1s0, Y2s0, Y1s1, Y2s1] (66 rows,
        # [x(2); state(64)]).  Our K=64 block order: xs, Y1s0, Y1s1,
        # Y2s0, Y2s1.  Y1 blocks arrive scaled by 32 -> fold 1/32 in.
        order = [0, 1, 3, 2, 4]
        scale = [1.0, 1 / 32.0, 1 / 32.0, 1.0, 1.0]
        Wm = np.zeros((5, 128, cols), dtype=np.float32)
        Wx10 = np.zeros((10, cols), dtype=np.float32)
        for i, j in enumerate(order):
            blk = W[j * 66:(j + 1) * 66]
            Wm[i, 0:64] = blk[2:66] * scale[i]
            Wm[i, 64:128] = blk[2:66] * scale[i]
            Wx10[2 * i:2 * i + 2] = blk[0:2]
        Wx = np.zeros((128, cols), dtype=np.float32)
        for base in (0, 32, 64, 96):
            Wx[base:base + 10] = Wx10
        return Wm.astype(bf16), Wx.astype(bf16)

    Wg_main, Wg_x = pack_w(np.asarray(W_gate, dtype=np.float32), 128)
    Wu_main, Wu_x = pack_w(np.asarray(W_update, dtype=np.float32), H)
    bg = np.ascontiguousarray(b_gate, dtype=np.float32).reshape(2 * H, 1)
    bu = np.ascontiguousarray(b_update, dtype=np.float32).reshape(H, 1)
    identb = np.eye(128).astype(bf16)

    # XALL selection: group g, local batch lb (b = 4g+lb), dest rows
    # lb*32 + [x(2), y1x_s0(2), y1x_s1(2), y2x_s0(2), y2x_s1(2)]
    P1 = np.zeros((4, 96, 128), dtype=np.float32)
    P2 = np.zeros((4, 2, RAG, 128), dtype=np.float32)
    for g in range(4):
        for lb in range(4):
            b = 4 * g + lb
            for k in range(2):
                P1[g, 2 * b + k, lb * 32 + k] = 1.0           # x
                P1[g, 32 + 2 * b + k, lb * 32 + 2 + k] = 1.0  # y1x s0
                P1[g, 64 + 2 * b + k, lb * 32 + 4 + k] = 1.0  # y1x s1
                P2[g, 0, 2 * b + k, lb * 32 + 6 + k] = 1.0    # y2x s0
                P2[g, 1, 2 * b + k, lb * 32 + 8 + k] = 1.0    # y2x s1
    P1 = P1.astype(bf16)
    P2 = P2.astype(bf16)

    in_maps = []
    for r in range(NCORES):
        n0 = r * NOWN
        stT32 = np.ascontiguousarray(
            state[:, n0:n0 + NOWN, :].transpose(0, 2, 1)
            .reshape(8, 128, NOWN))
        xTc = np.ascontiguousarray(
            x[:, n0:n0 + NOWN, :].transpose(0, 2, 1)
            .reshape(RAG, NOWN)).astype(bf16)
        in_maps.append({
            "Ts": np.ascontiguousarray(
                np.stack([support0[n0:n0 + NOWN, :].T,
                          support1[n0:n0 + NOWN, :].T])
                .astype(np.float32) * N).astype(fp8),
            "xs_main": xs_main,
            "xs_rag": xs_rag,
            "stateT": stT32.astype(bf16),
            "stateTf": stT32.astype(np.float32),
            "xT": xTc,
            "P1": P1, "P2": P2,
            "Wg_main": Wg_main, "Wg_x": Wg_x,
            "Wu_main": Wu_main, "Wu_x": Wu_x,
            "bg": bg, "bu": bu, "identb": identb,
        })
    return in_maps


def assemble_output(results):
    out = np.empty((B, N, H), dtype=np.float32)
    for r in range(NCORES):
        n0 = r * NOWN
        out[:, n0:n0 + NOWN, :] = results[r]["outT"].transpose(0, 2, 1)
    return out


def get_nc():
    if "nc" not in _NC_CACHE:
        _NC_CACHE["nc"] = build_nc()
    return _NC_CACHE["nc"]


def kernel(x, state, support0, support1, W_gate, b_gate, W_update, b_update):
    nc = get_nc()
    in_maps = prepare_in_maps(x, state, support0, support1,
                              W_gate, b_gate, W_update, b_update)
    prev = os.environ.get("BASS_NEVER_TRACE")
    os.environ["BASS_NEVER_TRACE"] = "1"
    try:
        res = run_bass_kernel_spmd(nc, in_maps, list(range(NCORES)),
                                   trace=False)
    finally:
        if prev is None:
            os.environ.pop("BASS_NEVER_TRACE", None)
        else:
            os.environ["BASS_NEVER_TRACE"] = prev
    return assemble_output(res.results)


# revision 33
# speedup vs baseline: 1.7072x; 1.0083x over previous
"""DCGRU cell (nn_DCGRUCell) Trainium2 Bass kernel, 8 NeuronCores — v3.

Sharding: node dim N=4096 split 8 ways (512/core); supports resident in
SBUF as fp8_e4m3 scaled by 4096 (values in [0,1]).  Diffusion matmuls
run fp8 (hop products staged/AllGathered as fp8 scaled by 32); the
undiffused feature block, W stage and gate math run bf16/fp32, so the
large-magnitude path keeps precision.  Scales are powers of two and are
folded into copy-out activations and host-side W rows (exact).

Node-major moving layout splits columns [B*H=1024 state | B*2=32 x];
the x-feature diffusion is identical for both GCNs and computed once.
Hop-1 outputs AllGather in two column halves so each collective hides
under the next compute phase.  Hop-1 results are double-written (fp8
AG input + bf16 local copy) so feature-major W blocks transpose-read
the local copy without serializing on the collective.  Hop-2 outputs
and all W-stage operands stay in SBUF.

kernel(**inputs) takes FULL inputs, returns FULL [16,4096,64] fp32.
"""
import os
import numpy as np

import concourse.mybir as mybir
import concourse.tile as tile
from concourse import bacc
from concourse.bass_utils import run_bass_kernel_spmd

F32 = mybir.dt.float32
BF16 = mybir.dt.bfloat16
FP8 = mybir.dt.float8e4
AF = mybir.ActivationFunctionType

NCORES = 8
B, N, H, DIN = 16, 4096, 64, 2
NOWN = N // NCORES          # 512 rows per core
NT = NOWN // 128            # 4 n-tiles per core
MT = N // 128               # 32 m-tiles (contraction)
MAIN = B * H                # 1024 state columns, batch-major
RAG = B * DIN               # 32 x columns, batch-major
HALF = 512                  # column half for AG chunking
SH_MAIN = NOWN * HALF       # elems of one support's half in a shard
SH_A = 2 * SH_MAIN          # chunk-A shard elems (both supports)
SH_RAGS = NOWN * RAG        # one support's ragged elems
SH_B = SH_A + 2 * SH_RAGS   # chunk-B shard elems
SH_C = NOWN * HALF          # cand chunk shard elems
GROUP = [list(range(NCORES))]

_NC_CACHE = {}


def build_nc():
    nc = bacc.Bacc("TRN2", target_bir_lowering=False, debug=False,
                   num_devices=NCORES)
    d = {}
    d["Ts"] = nc.dram_tensor("Ts", [2, N, NOWN], FP8, kind="ExternalInput")
    d["xs_main"] = nc.dram_tensor("xs_main", [N, MAIN], FP8,
                                  kind="ExternalInput")
    d["xs_rag"] = nc.dram_tensor("xs_rag", [N, RAG], FP8,
                                 kind="ExternalInput")
    d["stateT"] = nc.dram_tensor("stateT", [8, 128, NOWN], BF16,
                                 kind="ExternalInput")
    d["stateTf"] = nc.dram_tensor("stateTf", [8, 128, NOWN], F32,
                                  kind="ExternalInput")
    d["xT"] = nc.dram_tensor("xT", [RAG, NOWN], BF16, kind="ExternalInput")
    d["P1"] = nc.dram_tensor("P1", [4, 96, 128], BF16, kind="ExternalInput")
    d["P2"] = nc.dram_tensor("P2", [4, 2, RAG, 128], BF16,
                             kind="ExternalInput")
    d["Wg_main"] = nc.dram_tensor("Wg_main", [5, 128, 128], BF16,
                                  kind="ExternalInput")
    d["Wg_x"] = nc.dram_tensor("Wg_x", [128, 128], BF16,
                               kind="ExternalInput")
    d["Wu_main"] = nc.dram_tensor("Wu_main", [5, 128, H], BF16,
                                  kind="ExternalInput")
    d["Wu_x"] = nc.dram_tensor("Wu_x", [128, H], BF16, kind="ExternalInput")
    d["bg"] = nc.dram_tensor("bg", [2 * H, 1], F32, kind="ExternalInput")
    d["bu"] = nc.dram_tensor("bu", [H, 1], F32, kind="ExternalInput")
    d["identb"] = nc.dram_tensor("identb", [128, 128], BF16,
                                 kind="ExternalInput")
    d["outT"] = nc.dram_tensor("outT", [B, H, NOWN], F32,
                               kind="ExternalOutput")
    with tile.TileContext(nc) as tc:
        _emit(nc, tc, d)
    nc.compile()
    return nc


def _emit(nc, tc, d):
    import contextlib
    stack = contextlib.ExitStack()
    with stack:
        const = stack.enter_context(tc.tile_pool(name="const", bufs=1))
        res = stack.enter_context(tc.tile_pool(name="res", bufs=1))
        sb_mov = stack.enter_context(tc.tile_pool(name="mov", bufs=1))
        sb_ex = stack.enter_context(tc.tile_pool(name="ex", bufs=1))
        sb_y1 = stack.enter_context(tc.tile_pool(name="y1f", bufs=1))
        sb_y2 = stack.enter_context(tc.tile_pool(name="y2", bufs=1))
        sb_sm = stack.enter_context(tc.tile_pool(name="small", bufs=1))
        dram = stack.enter_context(
            tc.tile_pool(name="dram", bufs=1, space="DRAM"))
        psum = stack.enter_context(
            tc.tile_pool(name="psum", bufs=1, space="PSUM"))

        # ---- early loads: supports on the scalar HWDGE queue so the
        # sync queue serves the first sweep's moving tiles immediately
        CH = 4
        NCH = MT // CH
        Tch = {}
        for s in range(2):
            for k in range(NCH):
                Tch[(s, k)] = const.tile([128, CH, NOWN], FP8,
                                         name=f"T{s}_{k}")
        for k in range(NCH):
            for s in range(2):
                ts = d["Ts"].ap()[s].rearrange("(t p) n -> p t n", p=128)
                nc.scalar.dma_start(Tch[(s, k)][:],
                                    ts[:, k * CH:(k + 1) * CH, :])

        identb = const.tile([128, 128], BF16)
        xr_all = const.tile([128, MT, RAG], FP8)

        def emit_rag_loads():
            nc.sync.dma_start(identb[:], d["identb"].ap())
            nc.sync.dma_start(
                xr_all[:],
                d["xs_rag"].ap().rearrange("(m p) c -> p m c", p=128))

        def T_tile(s, m):
            return Tch[(s, m // CH)][:, m % CH, :]

        # ---- resident tiles (loads deferred; emitted after h1 below)
        S1 = res.tile([96, NOWN], BF16, name="S1")
        y2x = [res.tile([RAG, NOWN], BF16, name=f"y2x{s}") for s in range(2)]
        P1 = const.tile([96, 4, 128], BF16)
        P2 = const.tile([RAG, 4, 2, 128], BF16)
        stateT = [res.tile([128, NOWN], BF16, name=f"stT{p}")
                  for p in range(8)]
        stateTf = [res.tile([128, NOWN], F32, name=f"stTf{p}")
                   for p in range(8)]
        xall = [res.tile([128, NOWN], BF16, name=f"xall{g}")
                for g in range(4)]
        wg_m = const.tile([128, 5, 128], BF16)
        wu_m = const.tile([128, 5, H], BF16)
        wg_x = const.tile([128, 128], BF16)
        wu_x = const.tile([128, H], BF16)
        bg_t = const.tile([2 * H, 1], F32)
        bu_t = const.tile([H, 1], F32)
        rt = [res.tile([128, NOWN], F32, name=f"rt{p}") for p in range(8)]
        zs = [res.tile([128, NOWN], BF16, name=f"zs{p}") for p in range(8)]

        def emit_resident_loads():
            nc.sync.dma_start(S1[0:RAG, :], d["xT"].ap())
            for p in range(8):
                nc.sync.dma_start(stateT[p][:], d["stateT"].ap()[p])
                nc.sync.dma_start(stateTf[p][:], d["stateTf"].ap()[p])
            for j in range(5):
                nc.sync.dma_start(wg_m[:, j, :], d["Wg_main"].ap()[j])
                nc.sync.dma_start(wu_m[:, j, :], d["Wu_main"].ap()[j])
            nc.sync.dma_start(wg_x[:], d["Wg_x"].ap())
            nc.sync.dma_start(wu_x[:], d["Wu_x"].ap())
            nc.sync.dma_start(bg_t[:], d["bg"].ap())
            nc.sync.dma_start(bu_t[:], d["bu"].ap())
            for g in range(4):
                nc.sync.dma_start(P1[:, g, :], d["P1"].ap()[g])
                for s in range(2):
                    nc.sync.dma_start(P2[:, g, s, :], d["P2"].ap()[g, s])

        # ---------------- DRAM staging ----------------
        agin1A = dram.tile([SH_A], FP8, name="agin1A")
        agin1B = dram.tile([SH_B], FP8, name="agin1B")
        agout1A = dram.tile([NCORES * SH_A], FP8, name="agout1A",
                            addr_space="Shared")
        agout1B = dram.tile([NCORES * SH_B], FP8, name="agout1B",
                            addr_space="Shared")
        aginC = [dram.tile([SH_C], BF16, name=f"aginC{h}") for h in range(2)]
        agoutC = [dram.tile([NCORES * SH_C], BF16, name=f"agoutC{h}",
                            addr_space="Shared") for h in range(2)]
        agin2A = dram.tile([SH_A], FP8, name="agin2A")
        agin2B = dram.tile([SH_A], FP8, name="agin2B")
        agout2A = dram.tile([NCORES * SH_A], FP8, name="agout2A",
                            addr_space="Shared")
        agout2B = dram.tile([NCORES * SH_A], FP8, name="agout2B",
                            addr_space="Shared")
        y1loc = [[dram.tile([2 * NOWN * HALF], BF16, name=f"y1loc{i}{h}")
                  for h in range(2)] for i in range(2)]

        def agin_main(buf, s, t):
            a = buf.opt()
            off = s * SH_MAIN + t * 128 * HALF
            return a[off:off + 128 * HALF].rearrange("(p f) -> p f", f=HALF)

        def agin_rag(s, t):
            a = agin1B.opt()
            off = SH_A + s * SH_RAGS + t * 128 * RAG
            return a[off:off + 128 * RAG].rearrange("(p f) -> p f", f=RAG)

        def agout_mtile(buf, shard, s, m):
            a = buf.opt()
            q, t = m // NT, m % NT
            off = q * shard + s * SH_MAIN + t * 128 * HALF
            return a[off:off + 128 * HALF].rearrange("(p f) -> p f", f=HALF)

        def agout_rag_block(s, q):
            a = agout1B.opt()
            off = q * SH_B + SH_A + s * SH_RAGS
            return a[off:off + SH_RAGS].rearrange("(t p c) -> p t c", p=128,
                                                  c=RAG)

        def agoutC_cols(h, m):
            a = agoutC[h].opt()
            q, t = m // NT, m % NT
            v = a[q * SH_C:(q + 1) * SH_C].rearrange("(r c) -> r c", c=NOWN)
            return v[:, t * 128:(t + 1) * 128]

        def y1loc_tile(i, s, t, h):
            """[128, 512] write view: own node tile t, col half h."""
            a = y1loc[i][h].opt()
            off = s * NOWN * HALF + t * 128 * HALF
            return a[off:off + 128 * HALF].rearrange("(p f) -> p f", f=HALF)

        def y1loc_cols(i, s, p):
            """[512, 128] read view for transpose: bc cols p*128."""
            a = y1loc[i][p // 4].opt()
            v = a[s * NOWN * HALF:(s + 1) * NOWN * HALF].rearrange(
                "(n f) -> n f", f=HALF)
            return v[:, (p % 4) * 128:(p % 4 + 1) * 128]

        # ================= hop-1 sweep (one column half) =================
        def emit_h1_half(pid, mov_load, agin_buf, gi, h):
            """Y1[own, half cols] both supports; psum holds 4096*Y1.
            Writes fp8 32*Y1 to agin_buf and bf16 32*Y1 to y1loc."""
            ps = {}
            for s in range(2):
                for n in range(NT):
                    ps[(s, n)] = psum.tile([128, HALF], F32,
                                           name=f"ps{pid}{s}{n}",
                                           tag="acc", bufs=8)
            for m in range(MT):
                mv = sb_mov.tile([128, HALF],
                                 BF16 if gi == 1 else FP8,
                                 name=f"mv{pid}_{m}", tag="mov", bufs=6)
                mov_load(mv, m)
                for s in range(2):
                    for n in range(NT):
                        nc.tensor.matmul(
                            ps[(s, n)][:],
                            T_tile(s, m)[:, n * 128:(n + 1) * 128],
                            mv[:], start=(m == 0), stop=(m == MT - 1))
            for s in range(2):
                for n in range(NT):
                    ex8 = sb_ex.tile([128, HALF], FP8, name=f"e8{pid}{s}{n}",
                                     tag="ex8", bufs=6)
                    nc.scalar.mul(ex8[:], ps[(s, n)][:], 2.0 ** -7)
                    nc.scalar.dma_start(agin_main(agin_buf, s, n), ex8[:])
                    exb = sb_ex.tile([128, HALF], BF16, name=f"eb{pid}{s}{n}",
                                     tag="exb", bufs=6)
                    nc.vector.tensor_scalar_mul(exb[:], ps[(s, n)][:],
                                                2.0 ** -7)
                    nc.scalar.dma_start(y1loc_tile(gi, s, n, h), exb[:])

        # ================= hop-2 (one chunk, one support) ================
        def emit_h2_chunk(pid, s, agout_buf, shard, jbase, x0_tiles, y2_dst):
            """Y2^T[bc chunk, own] = 2*(A_s@Y1)^T - x0^T; stays in SBUF.
            Stationary fp8 is 32*Y1, T is 4096*A -> psum = 2^17*(A@Y1)."""
            ps = [psum.tile([128, NOWN], F32, name=f"ph2{pid}{jj}",
                            tag="acc", bufs=8) for jj in range(4)]
            for m in range(MT):
                st = sb_mov.tile([128, HALF], FP8, name=f"st{pid}_{m}",
                                 tag="st", bufs=8)
                nc.sync.dma_start(st[:], agout_mtile(agout_buf, shard, s, m))
                for jj in range(4):
                    nc.tensor.matmul(
                        ps[jj][:], st[:, jj * 128:(jj + 1) * 128],
                        T_tile(s, m), start=(m == 0), stop=(m == MT - 1))
            for jj in range(4):
                y2t = sb_y2.tile([128, NOWN], BF16, name=f"y2{pid}{jj}",
                                 tag="y2", bufs=16)
                nc.scalar.mul(y2t[:], ps[jj][:], 2.0 ** -16)
                nc.vector.tensor_sub(y2t[:], y2t[:], x0_tiles[jbase + jj][:])
                y2_dst[(s, jbase + jj)] = y2t

        # ================= W stage (one batch) =================
        def emit_w_gate(b, y1f, y2d):
            g, r2 = b // 4, (b % 4) * 32
            p, o = b // 2, (b % 2) * 64
            zr = psum.tile([128, NOWN], F32, name=f"zr{b}", tag="acc",
                           bufs=8)
            blocks = [stateT[p], y1f[(0, p)], y1f[(1, p)],
                      y2d[(0, p)], y2d[(1, p)]]
            for j in range(5):
                nc.tensor.matmul(zr[:], wg_m[o:o + 64, j, :],
                                 blocks[j][o:o + 64, :],
                                 start=(j == 0), stop=False)
            nc.tensor.matmul(zr[:], wg_x[r2:r2 + 10, :],
                             xall[g][r2:r2 + 10, :],
                             start=False, stop=True, skip_group_check=True,
                             tile_position=(r2, 0))
            ztmp = sb_sm.tile([128, NOWN], BF16, name=f"zt{b}", tag="ztmp",
                              bufs=2)
            nc.scalar.activation(ztmp[o:o + 64, :], zr[0:64, :], AF.Sigmoid,
                                 bias=bg_t[0:64])
            nc.scalar.activation(rt[p][o:o + 64, :], zr[64:128, :],
                                 AF.Sigmoid, bias=bg_t[64:128])
            nc.vector.tensor_mul(zs[p][o:o + 64, :], ztmp[o:o + 64, :],
                                 stateT[p][o:o + 64, :])
            nc.scalar.dma_start(
                aginC[b // 8].opt()[(b % 8) * 64 * NOWN:
                                    ((b % 8) + 1) * 64 * NOWN]
                .rearrange("(r c) -> r c", c=NOWN),
                zs[p][o:o + 64, :])

        def emit_w_update(b, y1f, y2d):
            g, r2 = b // 4, (b % 4) * 32
            p, o = b // 2, (b % 2) * 64
            hcp = psum.tile([H, NOWN], F32, name=f"hc{b}", tag="acc",
                            bufs=8)
            blocks = [zs[p], y1f[(0, p)], y1f[(1, p)],
                      y2d[(0, p)], y2d[(1, p)]]
            for j in range(5):
                nc.tensor.matmul(hcp[:], wu_m[o:o + 64, j, :],
                                 blocks[j][o:o + 64, :],
                                 start=(j == 0), stop=False)
            nc.tensor.matmul(hcp[:], wu_x[r2:r2 + 10, :],
                             xall[g][r2:r2 + 10, :],
                             start=False, stop=True, skip_group_check=True,
                             tile_position=(r2, 0))
            hc = sb_sm.tile([128, NOWN], F32, name=f"hcs{b}", tag="hc",
                            bufs=2)
            nc.scalar.activation(hc[o:o + 64, :], hcp[:], AF.Tanh,
                                 bias=bu_t[:])
            t1 = sb_sm.tile([128, NOWN], F32, name=f"t1{b}", tag="t1",
                            bufs=2)
            nc.vector.tensor_sub(t1[o:o + 64, :], stateTf[p][o:o + 64, :],
                                 hc[o:o + 64, :])
            nc.vector.tensor_mul(t1[o:o + 64, :], rt[p][o:o + 64, :],
                                 t1[o:o + 64, :])
            ot = sb_sm.tile([H, NOWN], F32, name=f"ot{b}", tag="ot", bufs=3)
            nc.vector.tensor_add(ot[:], hc[o:o + 64, :], t1[o:o + 64, :])
            nc.scalar.dma_start(d["outT"].ap()[b], ot[:])

        def ag(in_t, out_t):
            nc.gpsimd.collective_compute(
                "AllGather", mybir.AluOpType.bypass, replica_groups=GROUP,
                ins=[in_t.opt()], outs=[out_t.opt()])

        def emit_y1f(y1f, gi, prange):
            for s in range(2):
                for p in prange:
                    t = sb_y1.tile([128, NOWN], BF16,
                                   name=f"y1f{gi}{s}{p}", tag="y1f",
                                   bufs=16)
                    nc.scalar.dma_start(t[:], y1loc_cols(gi, s, p),
                                        transpose=True)
                    y1f[(s, p)] = t

        # ======================= GCN 1 =======================
        emit_h1_half(
            "g1h0",
            lambda mv, m: nc.sync.dma_start(
                mv[:], d["xs_main"].ap()[m * 128:(m + 1) * 128, 0:HALF]),
            agin1A, 0, 0)
        ag(agin1A, agout1A)
        emit_rag_loads()
        y1f1 = {}
        emit_y1f(y1f1, 0, range(4))

        # ragged x hop-1 (shared across GCNs): psum = 4096*(A_s @ x)^T
        psx = [psum.tile([RAG, NOWN], F32, name=f"psx{s}", tag="acc",
                         bufs=8) for s in range(2)]
        for m in range(MT):
            for s in range(2):
                nc.tensor.matmul(psx[s][:], xr_all[:, m, :], T_tile(s, m),
                                 start=(m == 0), stop=(m == MT - 1))
        for s in range(2):
            o = 32 + 32 * s
            nc.scalar.mul(S1[o:o + 32, :], psx[s][:], 2.0 ** -12)
            for t in range(NT):
                tp = psum.tile([128, RAG], BF16, name=f"tpx{s}{t}",
                               tag="acc", bufs=8)
                nc.tensor.transpose(tp[:],
                                    S1[o:o + 32, t * 128:(t + 1) * 128],
                                    identb[o:o + 32, o:o + 32])
                rnm = sb_sm.tile([128, RAG], FP8, name=f"rnm{s}{t}",
                                 tag="rnm", bufs=4)
                nc.scalar.mul(rnm[:], tp[:], 32.0)
                nc.scalar.dma_start(agin_rag(s, t), rnm[:])

        emit_h1_half(
            "g1h1",
            lambda mv, m: nc.sync.dma_start(
                mv[:], d["xs_main"].ap()[m * 128:(m + 1) * 128, HALF:MAIN]),
            agin1B, 0, 1)
        ag(agin1B, agout1B)

        emit_resident_loads()

        # hop-2 chunk A (cols 0..511 = batches 0..7)
        y2d1 = {}
        emit_h2_chunk("g1s0A", 0, agout1A, SH_A, 0, stateT, y2d1)
        emit_h2_chunk("g1s1A", 1, agout1A, SH_A, 0, stateT, y2d1)
        emit_y1f(y1f1, 0, range(4, 8))

        # x hop-2 (shared): y2x = 2*(A_s @ y1x)^T - x^T
        xr2 = [const.tile([128, MT, RAG], FP8, name=f"xr2_{s}")
               for s in range(2)]
        for s in range(2):
            for q in range(NCORES):
                nc.sync.dma_start(xr2[s][:, q * NT:(q + 1) * NT, :],
                                  agout_rag_block(s, q))
        psx2 = [psum.tile([RAG, NOWN], F32, name=f"psx2{s}", tag="acc",
                          bufs=8) for s in range(2)]
        for m in range(MT):
            for s in range(2):
                nc.tensor.matmul(psx2[s][:], xr2[s][:, m, :], T_tile(s, m),
                                 start=(m == 0), stop=(m == MT - 1))
        for s in range(2):
            nc.scalar.mul(y2x[s][:], psx2[s][:], 2.0 ** -16)
            nc.vector.tensor_sub(y2x[s][:], y2x[s][:], S1[0:RAG, :])
        # assemble XALL group tiles via PE permutation matmuls
        for g in range(4):
            psa = psum.tile([128, NOWN], F32, name=f"psa{g}", tag="acc",
                            bufs=8)
            nc.tensor.matmul(psa[:], P1[:, g, :], S1[:], start=True,
                             stop=False)
            nc.tensor.matmul(psa[:], P2[:, g, 0, :], y2x[0][:],
                             start=False, stop=False)
            nc.tensor.matmul(psa[:], P2[:, g, 1, :], y2x[1][:],
                             start=False, stop=True)
            nc.vector.tensor_copy(xall[g][:], psa[:])

        # W-gate batches 0..7 (needs chunk A), then cand AG half 0
        for b in range(8):
            emit_w_gate(b, y1f1, y2d1)
        ag(aginC[0], agoutC[0])

        # hop-2 chunk B (cols 512..1023 = batches 8..15)
        emit_h2_chunk("g1s0B", 0, agout1B, SH_B, 4, stateT, y2d1)
        emit_h2_chunk("g1s1B", 1, agout1B, SH_B, 4, stateT, y2d1)
        for b in range(8, 16):
            emit_w_gate(b, y1f1, y2d1)
        ag(aginC[1], agoutC[1])

        # ======================= GCN 2 =======================
        emit_h1_half(
            "g2h0",
            lambda mv, m: nc.sync.dma_start(mv[:], agoutC_cols(0, m),
                                            transpose=True),
            agin2A, 1, 0)
        ag(agin2A, agout2A)
        y1f2 = {}
        emit_y1f(y1f2, 1, range(4))
        emit_h1_half(
            "g2h1",
            lambda mv, m: nc.sync.dma_start(mv[:], agoutC_cols(1, m),
                                            transpose=True),
            agin2B, 1, 1)
        ag(agin2B, agout2B)

        y2d2 = {}
        emit_h2_chunk("g2s0A", 0, agout2A, SH_A, 0, zs, y2d2)
        emit_h2_chunk("g2s1A", 1, agout2A, SH_A, 0, zs, y2d2)
        emit_y1f(y1f2, 1, range(4, 8))
        for b in range(8):
            emit_w_update(b, y1f2, y2d2)
        emit_h2_chunk("g2s0B", 0, agout2B, SH_A, 4, zs, y2d2)
        emit_h2_chunk("g2s1B", 1, agout2B, SH_A, 4, zs, y2d2)
        for b in range(8, 16):
            emit_w_update(b, y1f2, y2d2)


def prepare_in_maps(x, state, support0, support1, W_gate, b_gate,
                    W_update, b_update):
    import ml_dtypes
    bf16 = ml_dtypes.bfloat16
    fp8 = ml_dtypes.float8_e4m3fn

    x = np.asarray(x, dtype=np.float32)
    state = np.asarray(state, dtype=np.float32)
    xs_main = np.ascontiguousarray(
        state.transpose(1, 0, 2).reshape(N, MAIN)).astype(fp8)
    xs_rag = np.ascontiguousarray(
        x.transpose(1, 0, 2).reshape(N, RAG)).astype(fp8)

    def pack_w(W, cols):
        # reference feats order: [xs, Y1s0, Y2s0, Y1s1, Y2s1] (66 rows,
        # [x(2); state(64)]).  Our K=64 block order: xs, Y1s0, Y1s1,
        # Y2s0, Y2s1.  Y1 blocks arrive scaled by 32 -> fold 1/32 in.
        order = [0, 1, 3, 2, 4]
        scale = [1.0, 1 / 32.0, 1 / 32.0, 1.0, 1.0]
        Wm = np.zeros((5, 128, cols), dtype=np.float32)
        Wx10 = np.zeros((10, cols), dtype=np.float32)
        for i, j in enumerate(order):
            blk = W[j * 66:(j + 1) * 66]
            Wm[i, 0:64] = blk[2:66] * scale[i]
            Wm[i, 64:128] = blk[2:66] * scale[i]
            Wx10[2 * i:2 * i + 2] = blk[0:2]
        Wx = np.zeros((128, cols), dtype=np.float32)
        for base in (0, 32, 64, 96):
            Wx[base:base + 10] = Wx10
        return Wm.astype(bf16), Wx.astype(bf16)

    Wg_main, Wg_x = pack_w(np.asarray(W_gate, dtype=np.float32), 128)
    Wu_main, Wu_x = pack_w(np.asarray(W_update, dtype=np.float32), H)
    bg = np.ascontiguousarray(b_gate, dtype=np.float32).reshape(2 * H, 1)
    bu = np.ascontiguousarray(b_update, dtype=np.float32).reshape(H, 1)
    identb = np.eye(128).astype(bf16)

    # XALL selection: group g, local batch lb (b = 4g+lb), dest rows
    # lb*32 + [x(2), y1x_s0(2), y1x_s1(2), y2x_s0(2), y2x_s1(2)]
    P1 = np.zeros((4, 96, 128), dtype=np.float32)
    P2 = np.zeros((4, 2, RAG, 128), dtype=np.float32)
    for g in range(4):
        for lb in range(4):
            b = 4 * g + lb
            for k in range(2):
                P1[g, 2 * b + k, lb * 32 + k] = 1.0           # x
                P1[g, 32 + 2 * b + k, lb * 32 + 2 + k] = 1.0  # y1x s0
                P1[g, 64 + 2 * b + k, lb * 32 + 4 + k] = 1.0  # y1x s1
                P2[g, 0, 2 * b + k, lb * 32 + 6 + k] = 1.0    # y2x s0
                P2[g, 1, 2 * b + k, lb * 32 + 8 + k] = 1.0    # y2x s1
    P1 = P1.astype(bf16)
    P2 = P2.astype(bf16)

    in_maps = []
    for r in range(NCORES):
        n0 = r * NOWN
        stT32 = np.ascontiguousarray(
            state[:, n0:n0 + NOWN, :].transpose(0, 2, 1)
            .reshape(8, 128, NOWN))
        xTc = np.ascontiguousarray(
            x[:, n0:n0 + NOWN, :].transpose(0, 2, 1)
            .reshape(RAG, NOWN)).astype(bf16)
        in_maps.append({
            "Ts": np.ascontiguousarray(
                np.stack([support0[n0:n0 + NOWN, :].T,
                          support1[n0:n0 + NOWN, :].T])
                .astype(np.float32) * N).astype(fp8),
            "xs_main": xs_main,
            "xs_rag": xs_rag,
            "stateT": stT32.astype(bf16),
            "stateTf": stT32.astype(np.float32),
            "xT": xTc,
            "P1": P1, "P2": P2,
            "Wg_main": Wg_main, "Wg_x": Wg_x,
            "Wu_main": Wu_main, "Wu_x": Wu_x,
            "bg": bg, "bu": bu, "identb": identb,
        })
    return in_maps


def assemble_output(results):
    out = np.empty((B, N, H), dtype=np.float32)
    for r in range(NCORES):
        n0 = r * NOWN
        out[:, n0:n0 + NOWN, :] = results[r]["outT"].transpose(0, 2, 1)
    return out


def get_nc():
    if "nc" not in _NC_CACHE:
        _NC_CACHE["nc"] = build_nc()
    return _NC_CACHE["nc"]


def kernel(x, state, support0, support1, W_gate, b_gate, W_update, b_update):
    nc = get_nc()
    in_maps = prepare_in_maps(x, state, support0, support1,
                              W_gate, b_gate, W_update, b_update)
    prev = os.environ.get("BASS_NEVER_TRACE")
    os.environ["BASS_NEVER_TRACE"] = "1"
    try:
        res = run_bass_kernel_spmd(nc, in_maps, list(range(NCORES)),
                                   trace=False)
    finally:
        if prev is None:
            os.environ.pop("BASS_NEVER_TRACE", None)
        else:
            os.environ["BASS_NEVER_TRACE"] = prev
    return assemble_output(res.results)
